# revision 1
# baseline (speedup 1.0000x reference)
"""Trainium2 Bass kernel: NeptuneTransformerEncoderLayer on 8 NeuronCores.

Sharding: batch(4) x seq-half(2) -> 8 cores, zero collectives.
Each core computes K/V for its batch's full 2048 tokens (10% redundant
FLOPs) and Q/attention/FFN for its own 1024 tokens.  The host permutes
each core's src so its query tokens are always rows [0:1024) -> one
uniform SPMD program.

Compute: fp16 operands on the PE (1 cycle/row), fp32 PSUM accumulation,
fp32 norm/softmax statistics.  All layout transposes ride the DMA xbar.
"""
import sys

for _p in ("/opt/trn_rl_repo", "/root/.axon_site/_ro/trn_rl_repo"):
    if _p not in sys.path:
        sys.path.insert(0, _p)

import numpy as np

import concourse.bass as bass
import concourse.mybir as mybir
import concourse.tile as tile
from concourse import bacc
from concourse import bass_utils

F16 = mybir.dt.float16
F32 = mybir.dt.float32
AF = mybir.ActivationFunctionType

P = 128
D = 1024            # d_model
DC = D // P         # 8 d-model chunks
NH = 16             # heads
HD = 64             # head dim
DFF = 4096
FC = DFF // P       # 32 ff chunks
S = 2048            # full sequence per batch
TQ = 1024           # query tokens per core
N_CORES = 8
EPS = 1e-5
BASE = 10000.0
ATT_DOWN = 1.0 / 64.0   # att_u eviction scale; undone by the 64/sum norm


def build_nc():
    nc = bacc.Bacc("TRN2", target_bir_lowering=False, debug=False,
                   num_devices=N_CORES)
    src = nc.dram_tensor("src", [S, D], F32, kind="ExternalInput")
    cos_t = nc.dram_tensor("cos_t", [P, S], F16, kind="ExternalInput")
    sin_t = nc.dram_tensor("sin_t", [P, S], F16, kind="ExternalInput")
    w_qkv = nc.dram_tensor("w_qkv", [3 * D, D], F32, kind="ExternalInput")
    w_out = nc.dram_tensor("w_out", [D, D], F32, kind="ExternalInput")
    w1 = nc.dram_tensor("w1", [DFF, D], F32, kind="ExternalInput")
    w2 = nc.dram_tensor("w2", [D, DFF], F32, kind="ExternalInput")
    w3 = nc.dram_tensor("w3", [DFF, D], F32, kind="ExternalInput")
    norm1_w = nc.dram_tensor("norm1_w", [D], F32, kind="ExternalInput")
    norm2_w = nc.dram_tensor("norm2_w", [D], F32, kind="ExternalInput")
    out = nc.dram_tensor("out", [TQ, D], F32, kind="ExternalOutput")

    with tile.TileContext(nc) as tc:
        emit(nc, tc, src, cos_t, sin_t, w_qkv, w_out, w1, w2, w3,
             norm1_w, norm2_w, out)
    nc.compile()
    return nc


def emit(nc, tc, src, cos_t, sin_t, w_qkv, w_out, w1, w2, w3,
         norm1_w, norm2_w, out):
    from contextlib import ExitStack

    ctx = ExitStack()
    with ctx:
        # pool groups with staged lifetimes (SBUF is 192KB/partition).
        # left stack: persist < p_att < p_qkv < phase pools
        # right stack: p_xnt, then p_mid/p_ht after it closes
        g_xnt = ctx.enter_context(ExitStack())   # closed after fused A/B/C
        g_qkv = ctx.enter_context(ExitStack())   # closed after fused A/B/C
        g_att = ctx.enter_context(ExitStack())   # closed after phase D
        persist = ctx.enter_context(tc.tile_pool(name="persist", bufs=1))
        p_att = g_att.enter_context(tc.tile_pool(name="p_att", bufs=1))
        p_xnt = g_xnt.enter_context(
            tc.tile_pool(name="p_xnt", bufs=1, side="right"))
        p_qkv = g_qkv.enter_context(tc.tile_pool(name="p_qkv", bufs=1))

        XNT = p_xnt.tile([P, DC, S], F16)        # x_norm1.T (d-major)
        C2 = p_xnt.tile([P, S], F16)
        S2 = p_xnt.tile([P, S], F16)
        QT = p_qkv.tile([P, DC, TQ], F16)        # roped q.T, parity-split rows
        KT = p_qkv.tile([P, DC, S], F16)         # roped k.T
        VA = p_qkv.tile([P, S // P, NH * 65], F16)  # v + ones col per head
        ATT = p_att.tile([P, DC, TQ], F16)       # normalized attn out .T
        eps_t = persist.tile([P, 1], F32)
        nc.vector.memset(eps_t[:], EPS)

        nc.sync.dma_start(C2[:], cos_t[:])
        nc.sync.dma_start(S2[:], sin_t[:])

        # ---- fused A/B/C: norm1+transpose, QKV+rope, V, attention ----
        # Emission order == engine stream order, arranged by data readiness:
        # src tiles, weight preps, then per head-pair (q, k, [V on pair 0],
        # attention) so the ACT-bound softmax overlaps the PE-bound QKV.
        with tc.tile_pool(name="pha", bufs=2) as pha, \
             tc.tile_pool(name="pha_s", bufs=4) as pha_s, \
             tc.tile_pool(name="phv_w", bufs=1) as phv_w, \
             tc.tile_pool(name="phb_w", bufs=2) as phb_w, \
             tc.tile_pool(name="phb", bufs=2) as phb, \
             tc.tile_pool(name="phc", bufs=2) as phc, \
             tc.tile_pool(name="ps_work", bufs=2, space="PSUM") as ps_work:

            def emit_a_ti(ti):
                st = pha.tile([P, D], F32, tag="src_in")
                nc.sync.dma_start(st[:], src[ti * P:(ti + 1) * P, :])
                ssq_a = pha_s.tile([P, 1], F32, tag="ssq_a")
                ssq_b = pha_s.tile([P, 1], F32, tag="ssq_b")
                for i, acc in ((0, ssq_a), (1, ssq_b)):
                    sq = ps_sq.tile([P, 512], F32, tag="sq")
                    nc.scalar.activation(sq[:], st[:, i * 512:(i + 1) * 512],
                                         AF.Square, accum_out=acc[:])
                ssq = pha_s.tile([P, 1], F32, tag="ssq")
                nc.vector.tensor_add(ssq[:], ssq_a[:], ssq_b[:])
                rms = pha_s.tile([P, 1], F32, tag="rms")
                nc.scalar.activation(rms[:], ssq[:], AF.Sqrt,
                                     bias=eps_t[:], scale=1.0 / D)
                rinv = pha_s.tile([P, 1], F32, tag="rinv")
                nc.vector.reciprocal(rinv[:], rms[:])
                xn = pha.tile([P, D], F16, tag="xn")
                nc.vector.tensor_scalar_mul(xn[:], st[:], rinv[:])
                nc.sync.dma_start(XNT[:, :, ti * P:(ti + 1) * P], xn[:],
                                  transpose=True)

            def prep_qk(j):
                # permuted row load (cast f32->f16 in SWDGE): partition
                # p = 64*h + 32*par + jp holds w_qkv row 128*j+64*h+2*jp+par
                w16 = phb_w.tile([P, D], F16, tag="w16")
                rp = w_qkv.ap()[j * P:(j + 1) * P, :].rearrange(
                    "(h jp par) d -> h par jp d", h=2, jp=32, par=2)
                for hh in range(2):
                    nc.gpsimd.dma_start(w16[hh * 64:(hh + 1) * 64, :], rp[hh])
                wT = phb_w.tile([P, DC, P], F16, tag="wT")
                nc.sync.dma_start(wT[:], w16[:], transpose=True)
                return wT

            def prep_v(j):
                w16 = phb_w.tile([P, D], F16, tag="w16")
                nc.gpsimd.dma_start(w16[:], w_qkv[(16 + j) * P:(17 + j) * P, :])
                wT = phv_w.tile([P, DC, P], F16, tag=f"wvT{j}")
                nc.sync.dma_start(wT[:], w16[:], transpose=True)
                return wT

            def qk_slice(j, wT, ts):
                sl = slice(ts * 512, ts * 512 + 512)
                pk = ps_work.tile([P, 512], F32, tag="work")
                for c in range(DC):
                    nc.tensor.matmul(pk[:], wT[:, c, :], XNT[:, c, sl],
                                     start=(c == 0), stop=(c == DC - 1))
                aa = phb.tile([P, 512], F16, tag="aa")
                nc.vector.tensor_mul(aa[:], pk[:], C2[:, sl])
                pp = phb.tile([P, 512], F16, tag="pp")
                nc.vector.tensor_mul(pp[:], pk[:], S2[:, sl])
                bb = phb.tile([P, 512], F16, tag="bb")
                for h0 in (0, 64):
                    nc.vector.tensor_copy(bb[h0:h0 + 32, :],
                                          pp[h0 + 32:h0 + 64, :])
                    nc.vector.tensor_copy(bb[h0 + 32:h0 + 64, :],
                                          pp[h0:h0 + 32, :])
                dst = (QT[:, j, sl] if j < 8 else KT[:, j - 8, sl])
                nc.vector.tensor_add(dst, aa[:], bb[:])

            def compute_qk(j, wT):
                for ts in range((TQ if j < 8 else S) // 512):
                    qk_slice(j, wT, ts)

            va3 = VA.rearrange("p t (h c) -> p t h c", c=65)

            def emit_v_ti(ti, wvTs):
                for half in range(2):
                    pvh = ps_work.tile([P, 512], F32, tag="work")
                    for j4 in range(4):
                        j = half * 4 + j4
                        for c in range(DC):
                            nc.tensor.matmul(
                                pvh[:, j4 * P:(j4 + 1) * P],
                                XNT[:, c, ti * P:(ti + 1) * P],
                                wvTs[j][:, c, :],
                                start=(c == 0), stop=(c == DC - 1))
                    hs = slice(half * 8, (half + 1) * 8)
                    nc.vector.memset(va3[:, ti, hs, 64], 1.0)
                    nc.vector.tensor_copy(
                        va3[:, ti, hs, 0:64],
                        pvh.rearrange("p (h c) -> p h c", c=64))

            def attend(h, ps_st, ps_pv):
                j, hb = h // 2, 64 * (h % 2)
                ppv = ps_pv.tile([65, TQ], F32, tag="ppv")
                for kc in range(S // P):
                    pst = ps_st.tile([P, TQ], F32, tag="pst")
                    for qs in range(2):
                        qsl = slice(qs * 512, qs * 512 + 512)
                        nc.tensor.matmul(
                            pst[:, qsl],
                            KT[hb:hb + 64, j, kc * P:(kc + 1) * P],
                            QT[hb:hb + 64, j, qsl],
                            start=True, stop=True)
                    pt16 = phc.tile([P, TQ], F16, tag="pt16")
                    nc.scalar.activation(pt16[:], pst[:], AF.Exp, scale=0.125)
                    for qs in range(2):
                        qsl = slice(qs * 512, qs * 512 + 512)
                        nc.tensor.matmul(ppv[:, qsl],
                                         VA[:, kc, 65 * h:65 * h + 65],
                                         pt16[:, qsl],
                                         start=(kc == 0),
                                         stop=(kc == S // P - 1))
                rec = phc.tile([1, TQ], F32, tag="rec")
                nc.vector.reciprocal(rec[:], ppv[64:65, :])
                r16 = phc.tile([1, TQ], F16, tag="r16")
                nc.vector.tensor_scalar_mul(r16[:], rec[:], 64.0)
                nrmS = phc.tile([64, TQ], F16, tag="nrmS")
                nc.gpsimd.partition_broadcast(nrmS[:], r16[:])
                att_u = phc.tile([64, TQ], F16, tag="att_u")
                nc.scalar.activation(att_u[:], ppv[0:64, :], AF.Copy,
                                     scale=ATT_DOWN)
                nc.vector.tensor_mul(ATT[hb:hb + 64, j, :], att_u[:], nrmS[:])

            with tc.tile_pool(name="ps_sq", bufs=2, space="PSUM") as ps_sq:
                for ti in range(4):
                    emit_a_ti(ti)
                wt0 = prep_qk(0)
                wv0 = prep_v(0)
                wv1 = prep_v(1)
                wt8 = prep_qk(8)
                for ti in range(4, S // P):
                    emit_a_ti(ti)
                wvTs = [wv0, wv1] + [prep_v(j) for j in range(2, 8)]

            with tc.tile_pool(name="ps_st", bufs=2, space="PSUM") as ps_st, \
                 tc.tile_pool(name="ps_pv", bufs=1, space="PSUM") as ps_pv:
                # pair 0: weave q/k slices with V rows in src-tile readiness
                # order so the PE is never waiting on late src tiles
                V = lambda ti: emit_v_ti(ti, wvTs)
                V(0)
                qk_slice(0, wt0, 0); V(1); V(2)
                qk_slice(0, wt0, 1); V(3); V(4)
                qk_slice(8, wt8, 0); V(5); V(6)
                qk_slice(8, wt8, 1); V(7); V(8)
                qk_slice(8, wt8, 2); V(9); V(10); V(11)
                qk_slice(8, wt8, 3)
                for ti in range(12, S // P):
                    V(ti)
                pend = {1: prep_qk(1), 9: prep_qk(9)}
                attend(0, ps_st, ps_pv)
                attend(1, ps_st, ps_pv)
                for hp in range(1, 8):
                    for jj in (hp, 8 + hp):
                        wT = pend.pop(jj, None)
                        if wT is None:
                            wT = prep_qk(jj)
                        compute_qk(jj, wT)
                    if hp < 7:
                        pend[hp + 1] = prep_qk(hp + 1)
                        pend[9 + hp] = prep_qk(9 + hp)
                    attend(2 * hp, ps_st, ps_pv)
                    attend(2 * hp + 1, ps_st, ps_pv)
        g_qkv.close()   # free QT / KT / VA
        g_xnt.close()   # free XNT / C2 / S2

        # late big tiles, allocated in the space freed by QKV
        p_mid = ctx.enter_context(tc.tile_pool(name="p_mid", bufs=1, side="right"))
        p_ht = ctx.enter_context(tc.tile_pool(name="p_ht", bufs=1, side="right"))
        X2 = p_mid.tile([P, TQ // P, D], F32)    # post-attn residual (t-major)
        XN2T = p_mid.tile([P, DC, TQ], F16)      # x_norm2.T
        HT = p_ht.tile([P, FC, TQ], F16)         # swiglu hidden .T

        # ---------------- phase D: out_proj + residual + rmsnorm2 --------
        with tc.tile_pool(name="phd_w", bufs=1) as phd_w, \
             tc.tile_pool(name="phd", bufs=3) as phd, \
             tc.tile_pool(name="phd_s", bufs=4) as phd_s, \
             tc.tile_pool(name="phd_ps", bufs=2, space="PSUM") as phd_ps:
            woTs = []
            for j in range(8):
                w16 = phd.tile([P, D], F16, tag="w16")
                nc.gpsimd.dma_start(w16[:], w_out[j * P:(j + 1) * P, :])
                wT = phd_w.tile([P, DC, P], F16, tag=f"woT{j}")
                nc.sync.dma_start(wT[:], w16[:], transpose=True)
                woTs.append(wT)
            for tb in range(TQ // P):
                py = phd_ps.tile([P, D], F32, tag="py")
                for j in range(8):
                    for c in range(DC):
                        nc.tensor.matmul(py[:, j * P:(j + 1) * P],
                                         ATT[:, c, tb * P:(tb + 1) * P],
                                         woTs[j][:, c, :],
                                         start=(c == 0), stop=(c == DC - 1))
                srcq = phd.tile([P, D], F32, tag="srcq")
                nc.sync.dma_start(srcq[:], src[tb * P:(tb + 1) * P, :])
                nc.vector.tensor_add(X2[:, tb, :], py[:], srcq[:])
                sq = phd.tile([P, D], F32, tag="sq")
                ssq = phd_s.tile([P, 1], F32, tag="ssq")
                nc.scalar.activation(sq[:], X2[:, tb, :], AF.Square,
                                     accum_out=ssq[:])
                rms = phd_s.tile([P, 1], F32, tag="rms")
                nc.scalar.activation(rms[:], ssq[:], AF.Sqrt,
                                     bias=eps_t[:], scale=1.0 / D)
                rinv = phd_s.tile([P, 1], F32, tag="rinv")
                nc.vector.reciprocal(rinv[:], rms[:])
                xn2 = phd.tile([P, D], F16, tag="xn2")
                nc.vector.tensor_scalar_mul(xn2[:], X2[:, tb, :], rinv[:])
                nc.sync.dma_start(XN2T[:, :, tb * P:(tb + 1) * P], xn2[:],
                                  transpose=True)
            # norm2_w is folded into w1/w3 on the host
        g_att.close()   # free ATT

        # ---------------- phase E: swiglu hidden ----------------
        with tc.tile_pool(name="phe", bufs=3) as phe, \
             tc.tile_pool(name="phe_ps", bufs=2, space="PSUM") as phe_ps:
            for fc in range(FC):
                w116 = phe.tile([P, D], F16, tag="w116")
                nc.gpsimd.dma_start(w116[:], w1[fc * P:(fc + 1) * P, :])
                w1T = phe.tile([P, DC, P], F16, tag="w1T")
                nc.sync.dma_start(w1T[:], w116[:], transpose=True)
                w316 = phe.tile([P, D], F16, tag="w316")
                nc.gpsimd.dma_start(w316[:], w3[fc * P:(fc + 1) * P, :])
                w3T = phe.tile([P, DC, P], F16, tag="w3T")
                nc.sync.dma_start(w3T[:], w316[:], transpose=True)
                pa = phe_ps.tile([P, TQ], F32, tag="pa")
                pb = phe_ps.tile([P, TQ], F32, tag="pb")
                for qs in range(2):
                    qsl = slice(qs * 512, qs * 512 + 512)
                    for c in range(DC):
                        nc.tensor.matmul(pa[:, qsl], w1T[:, c, :],
                                         XN2T[:, c, qsl],
                                         start=(c == 0), stop=(c == DC - 1))
                    for c in range(DC):
                        nc.tensor.matmul(pb[:, qsl], w3T[:, c, :],
                                         XN2T[:, c, qsl],
                                         start=(c == 0), stop=(c == DC - 1))
                sg = phe.tile([P, TQ], F16, tag="sg")
                nc.scalar.activation(sg[:], pa[:], AF.Sigmoid)
                sa = phe.tile([P, TQ], F16, tag="sa")
                nc.vector.tensor_mul(sa[:], sg[:], pa[:])
                nc.vector.tensor_mul(HT[:, fc, :], sa[:], pb[:])

        # ---------------- phase F: w2 + residual, output ----------------
        from concourse.masks import make_identity
        with tc.tile_pool(name="phf_c", bufs=1) as phf_c, \
             tc.tile_pool(name="phf", bufs=2) as phf, \
             tc.tile_pool(name="phf_y", bufs=1) as phf_y, \
             tc.tile_pool(name="phf_ps", bufs=2, space="PSUM") as phf_ps, \
             tc.tile_pool(name="phf_tp", bufs=4, space="PSUM") as phf_tp:
            ident = phf_c.tile([P, P], F32)
            make_identity(nc, ident)
            HF = FC // 2
            ofs = []
            for tb in range(TQ // P):
                of = phf_y.tile([P, D], F32, tag=f"of{tb}")
                ofs.append(of)
            for j in range(8):
                pz = phf_ps.tile([P, TQ], F32, tag="pz")
                for half in range(2):
                    w216 = phf.tile([P, DFF // 2], F16, tag="w216")
                    nc.gpsimd.dma_start(
                        w216[:],
                        w2[j * P:(j + 1) * P,
                           half * (DFF // 2):(half + 1) * (DFF // 2)])
                    w2T = phf.tile([P, HF, P], F16, tag="w2T")
                    nc.sync.dma_start(w2T[:], w216[:], transpose=True)
                    for f in range(HF):
                        fc = half * HF + f
                        for qs in range(2):
                            qsl = slice(qs * 512, qs * 512 + 512)
                            nc.tensor.matmul(pz[:, qsl], w2T[:, f, :],
                                             HT[:, fc, qsl],
                                             start=(fc == 0),
                                             stop=(fc == FC - 1))
                yj = phf.tile([P, TQ], F32, tag="yj")
                nc.scalar.activation(yj[:], pz[:], AF.Copy)
                # immediately fold this j-column back to token-major
                for tb in range(TQ // P):
                    ptp = phf_tp.tile([P, P], F32, tag="ptp")
                    nc.tensor.transpose(ptp[:], yj[:, tb * P:(tb + 1) * P],
                                        ident[:])
                    nc.vector.tensor_add(ofs[tb][:, j * P:(j + 1) * P],
                                         X2[:, tb, j * P:(j + 1) * P], ptp[:])
            for tb in range(TQ // P):
                nc.sync.dma_start(out[tb * P:(tb + 1) * P, :], ofs[tb][:])


_NC_CACHE = None


def _get_nc():
    global _NC_CACHE
    if _NC_CACHE is None:
        _NC_CACHE = build_nc()
    return _NC_CACHE


def _host_tables(positions_b, axis_scale):
    """Build parity-split fp16 cos/sin tables (128, S) for one batch."""
    coord = positions_b * axis_scale[None, :]              # (S, 4)
    invf = BASE ** (-(np.arange(0, 16, 2, dtype=np.float32) / 16.0))  # (8,)
    ang = coord[:, :, None] * invf[None, None, :]          # (S, 4, 8)
    ang = ang.reshape(S, 32).T                             # (32, S): r = 8a + j
    cos64 = np.concatenate([np.cos(ang), np.cos(ang)], axis=0)   # (64, S)
    sin64 = np.concatenate([np.sin(ang), -np.sin(ang)], axis=0)  # (64, S)
    c128 = np.concatenate([cos64, cos64], axis=0).astype(np.float16)
    s128 = np.concatenate([sin64, sin64], axis=0).astype(np.float16)
    return c128, s128


def build_in_maps(inputs):
    src = np.asarray(inputs["src"], dtype=np.float32)
    positions = np.asarray(inputs["positions"], dtype=np.float32)
    axis_scale = np.asarray(inputs["axis_scale"], np.float32)
    n1 = np.asarray(inputs["norm1_w"], np.float32)
    n2 = np.asarray(inputs["norm2_w"], np.float32)
    weights = {
        # rmsnorm gains are folded into the consuming projections
        "w_qkv": np.asarray(inputs["w_qkv"], np.float32) * n1[None, :],
        "w_out": np.asarray(inputs["w_out"], np.float32),
        "w1": np.asarray(inputs["w1"], np.float32) * n2[None, :],
        "w3": np.asarray(inputs["w3"], np.float32) * n2[None, :],
        "w2": np.asarray(inputs["w2"], np.float32),
        "norm1_w": n1, "norm2_w": n2,
    }
    in_maps = []
    for c in range(N_CORES):
        b, h = c // 2, c % 2
        sp = src[b]
        pp = positions[b]
        if h == 1:  # own half first
            sp = np.concatenate([sp[TQ:], sp[:TQ]], axis=0)
            pp = np.concatenate([pp[TQ:], pp[:TQ]], axis=0)
        ct, st = _host_tables(pp, axis_scale)
        m = {"src": np.ascontiguousarray(sp), "cos_t": ct, "sin_t": st}
        m.update(weights)
        in_maps.append(m)
    return in_maps


def kernel(src, positions, w_qkv, w_out, norm1_w, norm2_w, w1, w2, w3,
           axis_scale):
    src = np.asarray(src, dtype=np.float32)
    B = src.shape[0]
    in_maps = build_in_maps(dict(
        src=src, positions=positions, w_qkv=w_qkv, w_out=w_out,
        norm1_w=norm1_w, norm2_w=norm2_w, w1=w1, w2=w2, w3=w3,
        axis_scale=axis_scale))
    nc = _get_nc()
    res = bass_utils.run_bass_kernel_spmd(nc, in_maps,
                                          core_ids=list(range(N_CORES)))
    outp = np.zeros((B, S, D), np.float32)
    for c in range(N_CORES):
        b, h = c // 2, c % 2
        outp[b, h * TQ:(h + 1) * TQ, :] = res.results[c]["out"]
    return outp



# revision 2
# speedup vs baseline: 1.0095x; 1.0095x over previous
"""Trainium2 Bass kernel: NeptuneTransformerEncoderLayer on 8 NeuronCores.

Sharding: batch(4) x seq-half(2) -> 8 cores, zero collectives.  Each core
computes K/V for its batch's full 2048 tokens and Q/attention/FFN for its
own 1024 tokens (host permutes src so own queries are rows [0:1024)).

v2 vs baseline: fp8e4 DoubleRow matmuls (0.5 cyc/row) for QKV, PV,
out-proj and FFN down-proj; f16 for scores and FFN up-proj (precision).
All weights are host-side pre-transposed, pre-scaled and pre-quantized, so
no on-device weight transposes or SWDGE cast loads.  Scales: x_norm x8,
fp8 weights x64, v x8, hidden x16 (via w3 x16), probs exp-biased; inverse
scales fold into PSUM evictions and host weights.
"""
import sys

for _p in ("/opt/trn_rl_repo", "/root/.axon_site/_ro/trn_rl_repo"):
    if _p not in sys.path:
        sys.path.insert(0, _p)

import numpy as np
import ml_dtypes

import concourse.bass as bass
import concourse.mybir as mybir
import concourse.tile as tile
from concourse import bacc
from concourse import bass_utils

F8 = mybir.dt.float8e4
F16 = mybir.dt.float16
F32 = mybir.dt.float32
AF = mybir.ActivationFunctionType
DR = mybir.MatmulPerfMode.DoubleRow
NP8 = ml_dtypes.float8_e4m3

P = 128
D = 1024            # d_model
DC = D // P         # 8 d-model chunks
NH = 16             # heads
HD = 64             # head dim
DFF = 4096
FC = DFF // P       # 32 ff chunks
S = 2048            # full sequence per batch
TQ = 1024           # query tokens per core
N_CORES = 8
EPS = 1e-5
BASE = 10000.0

SX = 8.0            # x_norm fp8 scale
SW = 64.0           # fp8 weight scale
SV = 8.0            # v fp8 scale
SH = 8.0            # hidden fp8 scale (via w3)
EXP_BIAS = 1.0      # exp(0.125*s - EXP_BIAS); score stats: max 4.36, rowmax>=1.2


def build_nc():
    nc = bacc.Bacc("TRN2", target_bir_lowering=False, debug=False,
                   num_devices=N_CORES)
    src = nc.dram_tensor("src", [S, D], F32, kind="ExternalInput")
    cos_t = nc.dram_tensor("cos_t", [P, S], F16, kind="ExternalInput")
    sin_t = nc.dram_tensor("sin_t", [P, S], F16, kind="ExternalInput")
    wqkv8 = nc.dram_tensor("wqkv8", [P, 4, 2, 3 * D], F8, kind="ExternalInput")
    wo8 = nc.dram_tensor("wo8", [P, 4, 2, D], F8, kind="ExternalInput")
    w1t = nc.dram_tensor("w1t", [P, DC, DFF], F16, kind="ExternalInput")
    w3t = nc.dram_tensor("w3t", [P, DC, DFF], F16, kind="ExternalInput")
    w28 = nc.dram_tensor("w28", [P, FC // 2, 2, D], F8, kind="ExternalInput")
    out = nc.dram_tensor("out", [TQ, D], F32, kind="ExternalOutput")

    with tile.TileContext(nc) as tc:
        emit(nc, tc, src, cos_t, sin_t, wqkv8, wo8, w1t, w3t, w28, out)
    nc.compile()
    return nc


def emit(nc, tc, src, cos_t, sin_t, wqkv8, wo8, w1t, w3t, w28, out):
    from contextlib import ExitStack

    ctx = ExitStack()
    with ctx:
        # left stack: persist < p_att < p_qkv; right stack: p_xnt then p_mid
        g_xnt = ctx.enter_context(ExitStack())   # XNT8/C2/S2, freed after C
        g_qkv = ctx.enter_context(ExitStack())   # QT/KT/VA/weights, after C
        g_att = ctx.enter_context(ExitStack())   # ATT8, freed after D
        persist = ctx.enter_context(tc.tile_pool(name="persist", bufs=1))
        p_att = g_att.enter_context(tc.tile_pool(name="p_att", bufs=1))
        p_xnt = g_xnt.enter_context(
            tc.tile_pool(name="p_xnt", bufs=1, side="right"))
        p_qkv = g_qkv.enter_context(tc.tile_pool(name="p_qkv", bufs=1))

        XNT8 = p_xnt.tile([P, DC, S], F8)        # x_norm1.T fp8 (x8)
        C2 = p_xnt.tile([P, S], F16)
        S2 = p_xnt.tile([P, S], F16)
        QT = p_qkv.tile([P, DC, TQ], F16)        # roped q.T (true scale)
        KT = p_qkv.tile([P, DC, S], F16)         # roped k.T
        VA = p_qkv.tile([P, S // P, NH * 65], F8)   # v8 + ones col per head
        WQ8 = p_qkv.tile([P, 4, 2, 3 * D], F8)   # qkv weights (24KB/p)
        ATT = p_att.tile([P, DC, TQ], F8)        # attn out .T, fp8 (x8)
        eps_t = persist.tile([P, 1], F32)
        nc.vector.memset(eps_t[:], EPS)
        eps64_t = persist.tile([P, 1], F32)
        nc.vector.memset(eps64_t[:], EPS / (SX * SX))
        nbias_t = persist.tile([P, 1], F32)
        nc.vector.memset(nbias_t[:], -EXP_BIAS)

        nc.sync.dma_start(C2[:], cos_t[:])
        nc.sync.dma_start(S2[:], sin_t[:])
        nc.sync.dma_start(WQ8[:], wqkv8[:])

        va3 = VA.rearrange("p t (h c) -> p t h c", c=65)

        # ---- fused A/B/C: norm1+transpose, QKV+rope, V, attention ----
        with tc.tile_pool(name="pha", bufs=2) as pha, \
             tc.tile_pool(name="pha_t", bufs=3) as pha_t, \
             tc.tile_pool(name="pha_s", bufs=4) as pha_s, \
             tc.tile_pool(name="phb", bufs=3) as phb, \
             tc.tile_pool(name="phc", bufs=2) as phc, \
             tc.tile_pool(name="ps_work", bufs=2, space="PSUM") as ps_work:

            def emit_a_ti(ti):
                st = pha.tile([P, D], F32, tag="src_in")
                nc.sync.dma_start(st[:], src[ti * P:(ti + 1) * P, :])
                ssq_a = pha_s.tile([P, 1], F32, tag="ssq_a")
                ssq_b = pha_s.tile([P, 1], F32, tag="ssq_b")
                for i, acc in ((0, ssq_a), (1, ssq_b)):
                    sq = ps_sq.tile([P, 512], F32, tag="sq")
                    nc.scalar.activation(sq[:], st[:, i * 512:(i + 1) * 512],
                                         AF.Square, accum_out=acc[:])
                ssq = pha_s.tile([P, 1], F32, tag="ssq")
                nc.vector.tensor_add(ssq[:], ssq_a[:], ssq_b[:])
                # rms/SX = sqrt(ssq/(D*SX^2) + eps/SX^2) -> rinv = SX/rms
                rms = pha_s.tile([P, 1], F32, tag="rms")
                nc.scalar.activation(rms[:], ssq[:], AF.Sqrt,
                                     bias=eps64_t[:], scale=1.0 / (D * SX * SX))
                rinv = pha_s.tile([P, 1], F32, tag="rinv")
                nc.vector.reciprocal(rinv[:], rms[:])
                xn = pha.tile([P, D], F16, tag="xn")
                nc.vector.tensor_scalar_mul(xn[:], st[:], rinv[:])
                t16 = pha_t.tile([P, DC, P], F16, tag="t16")
                nc.sync.dma_start(t16[:], xn[:], transpose=True)
                nc.scalar.activation(XNT8[:, :, ti * P:(ti + 1) * P], t16[:],
                                     AF.Copy)

            def qk_slice(j, ts):
                # DoubleRow qkv projection for 512 tokens of q (j<8) / k
                sl = slice(ts * 512, ts * 512 + 512)
                pk = ps_work.tile([P, 512], F32, tag="work")
                for c2 in range(4):
                    nc.tensor.matmul(pk[:], WQ8[:, c2, :, j * P:(j + 1) * P],
                                     XNT8[:, 2 * c2:2 * c2 + 2, sl],
                                     start=(c2 == 0), stop=(c2 == 3),
                                     perf_mode=DR)
                # rope: evict to f16 (true scale), rotate pairs
                pk16 = phb.tile([P, 512], F16, tag="pk16")
                nc.vector.tensor_scalar_mul(pk16[:], pk[:], 1.0 / (SX * SW))
                aa = phb.tile([P, 512], F16, tag="aa")
                nc.vector.tensor_mul(aa[:], pk16[:], C2[:, sl])
                pp = phb.tile([P, 512], F16, tag="pp")
                nc.vector.tensor_mul(pp[:], pk16[:], S2[:, sl])
                bb = phb.tile([P, 512], F16, tag="bb")
                for h0 in (0, 64):
                    nc.vector.tensor_copy(bb[h0:h0 + 32, :],
                                          pp[h0 + 32:h0 + 64, :])
                    nc.vector.tensor_copy(bb[h0 + 32:h0 + 64, :],
                                          pp[h0:h0 + 32, :])
                dst = (QT[:, j, sl] if j < 8 else KT[:, j - 8, sl])
                nc.vector.tensor_add(dst, aa[:], bb[:])

            def emit_v_ti(ti):
                for half in range(2):
                    pvh = ps_work.tile([P, 512], F32, tag="work")
                    for j4 in range(4):
                        j = 16 + half * 4 + j4
                        for c2 in range(4):
                            nc.tensor.matmul(
                                pvh[:, j4 * P:(j4 + 1) * P],
                                XNT8[:, 2 * c2:2 * c2 + 2,
                                     ti * P:(ti + 1) * P],
                                WQ8[:, c2, :, j * P:(j + 1) * P],
                                start=(c2 == 0), stop=(c2 == 3),
                                perf_mode=DR)
                    hs = slice(half * 8, (half + 1) * 8)
                    nc.vector.memset(va3[:, ti, hs, 64], 1.0)
                    # v8 = pvh * SV/(SX*SW)
                    nc.scalar.activation(
                        va3[:, ti, hs, 0:64],
                        pvh.rearrange("p (h c) -> p h c", c=64),
                        AF.Copy, scale=SV / (SX * SW))

            def attend(h, ps_st, ps_pv):
                j, hb = h // 2, 64 * (h % 2)
                ppv = ps_pv.tile([65, TQ], F32, tag="ppv")
                pt8 = phc.tile([P, 2, TQ], F8, tag="pt8")
                for kc in range(S // P):
                    pst = ps_st.tile([P, TQ], F32, tag="pst")
                    for qs in range(2):
                        qsl = slice(qs * 512, qs * 512 + 512)
                        nc.tensor.matmul(
                            pst[:, qsl],
                            KT[hb:hb + 64, j, kc * P:(kc + 1) * P],
                            QT[hb:hb + 64, j, qsl],
                            start=True, stop=True)
                    nc.scalar.activation(pt8[:, kc % 2, :], pst[:], AF.Exp,
                                         scale=0.125, bias=nbias_t[:])
                    if kc % 2 == 1:
                        for qs in range(2):
                            qsl = slice(qs * 512, qs * 512 + 512)
                            nc.tensor.matmul(
                                ppv[:, qsl],
                                VA[:, kc - 1:kc + 1, 65 * h:65 * h + 65],
                                pt8[:, :, qsl],
                                start=(kc == 1), stop=(kc == S // P - 1),
                                perf_mode=DR)
                        pt8 = phc.tile([P, 2, TQ], F8, tag="pt8")
                rec = phc.tile([1, TQ], F32, tag="rec")
                nc.vector.reciprocal(rec[:], ppv[64:65, :])
                nrmS = phc.tile([64, TQ], F32, tag="nrmS")
                nc.gpsimd.partition_broadcast(nrmS[:], rec[:])
                nc.vector.tensor_mul(ATT[hb:hb + 64, j, :], ppv[0:64, :],
                                     nrmS[:])

            with tc.tile_pool(name="ps_sq", bufs=2, space="PSUM") as ps_sq:
                for ti in range(4):
                    emit_a_ti(ti)
                for ti in range(4, S // P):
                    emit_a_ti(ti)

            with tc.tile_pool(name="ps_st", bufs=2, space="PSUM") as ps_st, \
                 tc.tile_pool(name="ps_pv", bufs=1, space="PSUM") as ps_pv:
                # V rows in src-tile readiness order, woven with q/k slices
                emit_v_ti(0)
                qk_slice(0, 0); emit_v_ti(1); emit_v_ti(2)
                qk_slice(0, 1); emit_v_ti(3); emit_v_ti(4)
                qk_slice(8, 0); emit_v_ti(5); emit_v_ti(6)
                qk_slice(8, 1); emit_v_ti(7); emit_v_ti(8)
                qk_slice(8, 2); emit_v_ti(9); emit_v_ti(10); emit_v_ti(11)
                qk_slice(8, 3)
                for ti in range(12, S // P):
                    emit_v_ti(ti)
                attend(0, ps_st, ps_pv)
                attend(1, ps_st, ps_pv)
                for hp in range(1, 8):
                    for jj in (hp, 8 + hp):
                        for ts in range(2 if jj < 8 else 4):
                            qk_slice(jj, ts)
                    attend(2 * hp, ps_st, ps_pv)
                    attend(2 * hp + 1, ps_st, ps_pv)
        g_qkv.close()   # free QT / KT / VA / WQ8 / WO8 -- WO8 needed in D!
        g_xnt.close()   # free XNT8 / C2 / S2

        # late big tiles, allocated in the space freed by QKV
        p_mid = ctx.enter_context(tc.tile_pool(name="p_mid", bufs=1,
                                               side="right"))
        p_ht = ctx.enter_context(tc.tile_pool(name="p_ht", bufs=1,
                                              side="right"))
        X2 = p_mid.tile([P, TQ // P, D], F32)    # post-attn residual (t-major)
        XN2T = p_mid.tile([P, DC, TQ], F16)      # x_norm2.T
        HT8 = p_ht.tile([P, FC, TQ], F8)         # swiglu hidden .T (x16)

        # ---------------- phase D: out_proj + residual + rmsnorm2 --------
        with tc.tile_pool(name="phd_w", bufs=1) as phd_w, \
             tc.tile_pool(name="phd", bufs=3) as phd, \
             tc.tile_pool(name="phd_s", bufs=4) as phd_s, \
             tc.tile_pool(name="phd_ps", bufs=2, space="PSUM") as phd_ps, \
             tc.tile_pool(name="phd_sq", bufs=2, space="PSUM") as phd_sq:
            WO8b = phd_w.tile([P, 4, 2, D], F8)
            nc.sync.dma_start(WO8b[:], wo8[:])
            for tb in range(TQ // P):
                py = phd_ps.tile([P, D], F32, tag="py")
                for j in range(8):
                    for c2 in range(4):
                        nc.tensor.matmul(py[:, j * P:(j + 1) * P],
                                         ATT[:, 2 * c2:2 * c2 + 2,
                                             tb * P:(tb + 1) * P],
                                         WO8b[:, c2, :, j * P:(j + 1) * P],
                                         start=(c2 == 0), stop=(c2 == 3),
                                         perf_mode=DR)
                srcq = phd.tile([P, D], F32, tag="srcq")
                nc.sync.dma_start(srcq[:], src[tb * P:(tb + 1) * P, :])
                nc.vector.scalar_tensor_tensor(
                    X2[:, tb, :], py[:], 1.0 / (SV * SW), srcq[:],
                    op0=mybir.AluOpType.mult, op1=mybir.AluOpType.add)
                sq = phd_sq.tile([P, D], F32, tag="sq")
                ssq = phd_s.tile([P, 1], F32, tag="ssq")
                nc.scalar.activation(sq[:], X2[:, tb, :], AF.Square,
                                     accum_out=ssq[:])
                rms = phd_s.tile([P, 1], F32, tag="rms")
                nc.scalar.activation(rms[:], ssq[:], AF.Sqrt,
                                     bias=eps_t[:], scale=1.0 / D)
                rinv = phd_s.tile([P, 1], F32, tag="rinv")
                nc.vector.reciprocal(rinv[:], rms[:])
                xn2 = phd.tile([P, D], F16, tag="xn2")
                nc.vector.tensor_scalar_mul(xn2[:], X2[:, tb, :], rinv[:])
                nc.sync.dma_start(XN2T[:, :, tb * P:(tb + 1) * P], xn2[:],
                                  transpose=True)
            # norm2_w is folded into w1/w3 on the host
        g_att.close()   # free ATT

        # ---------------- phase E: swiglu hidden (f16) ----------------
        with tc.tile_pool(name="phe_w", bufs=3) as phe_w, \
             tc.tile_pool(name="phe", bufs=3) as phe, \
             tc.tile_pool(name="phe_ps", bufs=2, space="PSUM") as phe_ps:
            for fc in range(FC):
                w1c = phe_w.tile([P, DC, P], F16, tag="w1c")
                nc.sync.dma_start(w1c[:], w1t[:, :, fc * P:(fc + 1) * P])
                w3c = phe_w.tile([P, DC, P], F16, tag="w3c")
                nc.sync.dma_start(w3c[:], w3t[:, :, fc * P:(fc + 1) * P])
                pa = phe_ps.tile([P, TQ], F32, tag="pa")
                pb = phe_ps.tile([P, TQ], F32, tag="pb")
                for qs in range(2):
                    qsl = slice(qs * 512, qs * 512 + 512)
                    for c in range(DC):
                        nc.tensor.matmul(pa[:, qsl], w1c[:, c, :],
                                         XN2T[:, c, qsl],
                                         start=(c == 0), stop=(c == DC - 1))
                    for c in range(DC):
                        nc.tensor.matmul(pb[:, qsl], w3c[:, c, :],
                                         XN2T[:, c, qsl],
                                         start=(c == 0), stop=(c == DC - 1))
                sg = phe.tile([P, TQ], F16, tag="sg")
                nc.scalar.activation(sg[:], pa[:], AF.Sigmoid)
                sa = phe.tile([P, TQ], F16, tag="sa")
                nc.vector.tensor_mul(sa[:], sg[:], pa[:])
                nc.vector.tensor_mul(HT8[:, fc, :], sa[:], pb[:])

        # ---------------- phase F: w2 + residual, output ----------------
        from concourse.masks import make_identity
        with tc.tile_pool(name="phf_c", bufs=1) as phf_c, \
             tc.tile_pool(name="phf_w", bufs=1) as phf_w, \
             tc.tile_pool(name="phf", bufs=2) as phf, \
             tc.tile_pool(name="phf_y", bufs=1) as phf_y, \
             tc.tile_pool(name="phf_ps", bufs=2, space="PSUM") as phf_ps, \
             tc.tile_pool(name="phf_tp", bufs=4, space="PSUM") as phf_tp:
            ident = phf_c.tile([P, P], F32)
            make_identity(nc, ident)
            W28b = phf_w.tile([P, FC // 2, 2, D], F8)
            nc.sync.dma_start(W28b[:], w28[:])
            ofs = []
            for tb in range(TQ // P):
                of = phf_y.tile([P, D], F32, tag=f"of{tb}")
                ofs.append(of)
            for j in range(8):
                pz = phf_ps.tile([P, TQ], F32, tag="pz")
                for fp in range(FC // 2):
                    for qs in range(2):
                        qsl = slice(qs * 512, qs * 512 + 512)
                        nc.tensor.matmul(pz[:, qsl],
                                         W28b[:, fp, :, j * P:(j + 1) * P],
                                         HT8[:, 2 * fp:2 * fp + 2, qsl],
                                         start=(fp == 0),
                                         stop=(fp == FC // 2 - 1),
                                         perf_mode=DR)
                yj = phf.tile([P, TQ], F32, tag="yj")
                nc.scalar.activation(yj[:], pz[:], AF.Copy,
                                     scale=1.0 / (SH * SW))
                # immediately fold this j-column back to token-major
                for tb in range(TQ // P):
                    ptp = phf_tp.tile([P, P], F32, tag="ptp")
                    nc.tensor.transpose(ptp[:], yj[:, tb * P:(tb + 1) * P],
                                        ident[:])
                    nc.vector.tensor_add(ofs[tb][:, j * P:(j + 1) * P],
                                         X2[:, tb, j * P:(j + 1) * P], ptp[:])
            for tb in range(TQ // P):
                nc.sync.dma_start(out[tb * P:(tb + 1) * P, :], ofs[tb][:])


_NC_CACHE = None


def _get_nc():
    global _NC_CACHE
    if _NC_CACHE is None:
        _NC_CACHE = build_nc()
    return _NC_CACHE


def _host_tables(positions_b, axis_scale):
    """Build parity-split fp16 cos/sin tables (128, S) for one batch."""
    coord = positions_b * axis_scale[None, :]              # (S, 4)
    invf = BASE ** (-(np.arange(0, 16, 2, dtype=np.float32) / 16.0))  # (8,)
    ang = coord[:, :, None] * invf[None, None, :]          # (S, 4, 8)
    ang = ang.reshape(S, 32).T                             # (32, S): r = 8a + j
    cos64 = np.concatenate([np.cos(ang), np.cos(ang)], axis=0)   # (64, S)
    sin64 = np.concatenate([np.sin(ang), -np.sin(ang)], axis=0)  # (64, S)
    c128 = np.concatenate([cos64, cos64], axis=0).astype(np.float16)
    s128 = np.concatenate([sin64, sin64], axis=0).astype(np.float16)
    return c128, s128


def _prep_weights(inputs):
    """Pre-transpose / permute / scale / quantize all weights on the host."""
    n1 = np.asarray(inputs["norm1_w"], np.float32)
    n2 = np.asarray(inputs["norm2_w"], np.float32)
    w_qkv = np.asarray(inputs["w_qkv"], np.float32) * n1[None, :]
    w_out = np.asarray(inputs["w_out"], np.float32)
    w1 = np.asarray(inputs["w1"], np.float32) * n2[None, :]
    w3 = np.asarray(inputs["w3"], np.float32) * n2[None, :]
    w2 = np.asarray(inputs["w2"], np.float32)

    # qkv row permutation: for j<16 (q,k) parity split rows; v plain
    rows = np.zeros(3 * D, np.int64)
    for j in range(24):
        for col in range(P):
            if j < 16:
                h, r = divmod(col, 64)
                par, jp = divmod(r, 32)
                rows[j * P + col] = 128 * j + 64 * h + 2 * jp + par
            else:
                rows[j * P + col] = j * P + col
    wq_perm = w_qkv[rows, :]                   # (3D, D) row-permuted
    # wqkv8[p, c2, e, jf]  = wq_perm[jf, (2*c2+e)*128 + p] * SW
    wqkv8 = (wq_perm.T.reshape(4, 2, P, 3 * D).transpose(2, 0, 1, 3)
             * SW).astype(NP8)
    wqkv8 = np.ascontiguousarray(wqkv8)
    # wo8[p, c2, e, f] = w_out[f, (2*c2+e)*128 + p] * SW
    wo8 = (w_out.T.reshape(4, 2, P, D).transpose(2, 0, 1, 3) * SW).astype(NP8)
    wo8 = np.ascontiguousarray(wo8)
    # w1t[p, c, f] = w1[f, c*128 + p]
    w1t = np.ascontiguousarray(
        w1.T.reshape(DC, P, DFF).transpose(1, 0, 2)).astype(np.float16)
    w3t = np.ascontiguousarray(
        w3.T.reshape(DC, P, DFF).transpose(1, 0, 2) * SH).astype(np.float16)
    # w28[p, fp, e, f] = w2[f, (2*fp+e)*128 + p] * SW
    w28 = (w2.T.reshape(FC // 2, 2, P, D).transpose(2, 0, 1, 3)
           * SW).astype(NP8)
    w28 = np.ascontiguousarray(w28)
    return {"wqkv8": wqkv8, "wo8": wo8, "w1t": w1t, "w3t": w3t, "w28": w28}


def build_in_maps(inputs):
    src = np.asarray(inputs["src"], dtype=np.float32)
    positions = np.asarray(inputs["positions"], dtype=np.float32)
    axis_scale = np.asarray(inputs["axis_scale"], np.float32)
    weights = _prep_weights(inputs)
    in_maps = []
    for c in range(N_CORES):
        b, h = c // 2, c % 2
        sp = src[b]
        pp = positions[b]
        if h == 1:  # own half first
            sp = np.concatenate([sp[TQ:], sp[:TQ]], axis=0)
            pp = np.concatenate([pp[TQ:], pp[:TQ]], axis=0)
        ct, st = _host_tables(pp, axis_scale)
        m = {"src": np.ascontiguousarray(sp), "cos_t": ct, "sin_t": st}
        m.update(weights)
        in_maps.append(m)
    return in_maps


def kernel(src, positions, w_qkv, w_out, norm1_w, norm2_w, w1, w2, w3,
           axis_scale):
    src = np.asarray(src, dtype=np.float32)
    B = src.shape[0]
    in_maps = build_in_maps(dict(
        src=src, positions=positions, w_qkv=w_qkv, w_out=w_out,
        norm1_w=norm1_w, norm2_w=norm2_w, w1=w1, w2=w2, w3=w3,
        axis_scale=axis_scale))
    nc = _get_nc()
    res = bass_utils.run_bass_kernel_spmd(nc, in_maps,
                                          core_ids=list(range(N_CORES)))
    outp = np.zeros((B, S, D), np.float32)
    for c in range(N_CORES):
        b, h = c // 2, c % 2
        outp[b, h * TQ:(h + 1) * TQ, :] = res.results[c]["out"]
    return outp


# revision 3
# speedup vs baseline: 1.0289x; 1.0192x over previous
"""Trainium2 Bass kernel: NeptuneTransformerEncoderLayer on 8 NeuronCores.

Sharding: batch(4) x seq-half(2) -> 8 cores, zero collectives.  Each core
computes K/V for its batch's full 2048 tokens and Q/attention/FFN for its
own 1024 tokens (host permutes src so own queries are rows [0:1024)).

v2: fp8e4 DoubleRow matmuls (0.5 cyc/row) for QKV, PV, out-proj and FFN
down-proj; f16 for scores and FFN up-proj (precision).  Weights are
host-side pre-transposed/pre-scaled/pre-quantized (no on-device weight
transposes).  The ACT-bound attention is split into two 512-query blocks
and the second block's attention is interleaved with the first block's
PE-bound FFN so both engines stay busy.  Scores PSUM tiles pair kc chunks
so exp keeps 1024-wide slices; PV emission is skewed one kc-pair behind
scores so the PE never waits on exp.
"""
import sys

for _p in ("/opt/trn_rl_repo", "/root/.axon_site/_ro/trn_rl_repo"):
    if _p not in sys.path:
        sys.path.insert(0, _p)

import numpy as np
import ml_dtypes

import concourse.bass as bass
import concourse.mybir as mybir
import concourse.tile as tile
from concourse import bacc
from concourse import bass_utils

F8 = mybir.dt.float8e4
F16 = mybir.dt.float16
F32 = mybir.dt.float32
AF = mybir.ActivationFunctionType
DR = mybir.MatmulPerfMode.DoubleRow
NP8 = ml_dtypes.float8_e4m3

P = 128
D = 1024            # d_model
DC = D // P         # 8 d-model chunks
NH = 16             # heads
HD = 64             # head dim
DFF = 4096
FC = DFF // P       # 32 ff chunks
S = 2048            # full sequence per batch
TQ = 1024           # query tokens per core
QB = 512            # query pipeline block
N_CORES = 8
EPS = 1e-5
BASE = 10000.0

SX = 8.0            # x_norm fp8 scale
SW = 64.0           # fp8 weight scale
SV = 8.0            # v fp8 scale
SH = 8.0            # hidden fp8 scale (via w3)
EXP_BIAS = 1.0      # exp(0.125*s - EXP_BIAS); score stats: max 4.36, rowmax>=1.2


def build_nc():
    nc = bacc.Bacc("TRN2", target_bir_lowering=False, debug=False,
                   num_devices=N_CORES)
    src = nc.dram_tensor("src", [S, D], F32, kind="ExternalInput")
    cos_t = nc.dram_tensor("cos_t", [P, S], F16, kind="ExternalInput")
    sin_t = nc.dram_tensor("sin_t", [P, S], F16, kind="ExternalInput")
    wqkv8 = nc.dram_tensor("wqkv8", [P, 4, 2, 3 * D], F8, kind="ExternalInput")
    wo8 = nc.dram_tensor("wo8", [P, 4, 2, D], F8, kind="ExternalInput")
    w13t = nc.dram_tensor("w13t", [P, DC, 2, DFF], F16, kind="ExternalInput")
    w28 = nc.dram_tensor("w28", [P, FC // 2, 2, D], F8, kind="ExternalInput")
    out = nc.dram_tensor("out", [TQ, D], F32, kind="ExternalOutput")

    with tile.TileContext(nc) as tc:
        emit(nc, tc, src, cos_t, sin_t, wqkv8, wo8, w13t, w28, out)
    nc.compile()
    return nc


def emit(nc, tc, src, cos_t, sin_t, wqkv8, wo8, w13t, w28, out):
    from contextlib import ExitStack

    ctx = ExitStack()
    with ctx:
        g_xnt = ctx.enter_context(ExitStack())   # XNT8/C2/S2/WQ8, freed post-B
        persist = ctx.enter_context(tc.tile_pool(name="persist", bufs=1))
        p_att = ctx.enter_context(tc.tile_pool(name="p_att", bufs=1))
        p_xnt = g_xnt.enter_context(
            tc.tile_pool(name="p_xnt", bufs=1, side="right"))
        p_qkv = ctx.enter_context(tc.tile_pool(name="p_qkv", bufs=1))

        XNT8 = p_xnt.tile([P, DC, S], F8)        # x_norm1.T fp8 (x8)
        WQ8 = p_xnt.tile([P, 4, 2, 3 * D], F8)   # qkv weights (24KB/p)
        C2 = p_xnt.tile([P, S], F16)
        S2 = p_xnt.tile([P, S], F16)
        QT = p_qkv.tile([P, DC, TQ], F16)        # roped q.T (true scale)
        KT = p_qkv.tile([P, DC, S], F16)         # roped k.T
        VA = p_qkv.tile([P, S // P, NH * 65], F8)   # v8 + ones col per head
        ATT = p_att.tile([P, DC, TQ], F8)        # attn out .T, fp8 (x8)
        WO8b = persist.tile([P, 4, 2, D], F8)
        eps_t = persist.tile([P, 1], F32)
        nc.vector.memset(eps_t[:], EPS)
        eps64_t = persist.tile([P, 1], F32)
        nc.vector.memset(eps64_t[:], EPS / (SX * SX))
        nbias_t = persist.tile([P, 1], F32)
        nc.vector.memset(nbias_t[:], -EXP_BIAS)

        nc.sync.dma_start(C2[:], cos_t[:])
        nc.sync.dma_start(S2[:], sin_t[:])
        nc.scalar.dma_start(WQ8[:], wqkv8[:])
        nc.scalar.dma_start(WO8b[:], wo8[:])

        va3 = VA.rearrange("p t (h c) -> p t h c", c=65)

        # long-lived working pools first (LIFO stack: closed last)
        phc = ctx.enter_context(tc.tile_pool(name="phc", bufs=2))
        phn = ctx.enter_context(tc.tile_pool(name="phn", bufs=1))
        phd = ctx.enter_context(tc.tile_pool(name="phd", bufs=2))
        phd_s = ctx.enter_context(tc.tile_pool(name="phd_s", bufs=4))
        phe_w = ctx.enter_context(tc.tile_pool(name="phe_w", bufs=3))
        phe = ctx.enter_context(tc.tile_pool(name="phe", bufs=2))
        phf = ctx.enter_context(tc.tile_pool(name="phf", bufs=2))
        phf_w = ctx.enter_context(tc.tile_pool(name="phf_w", bufs=2))
        # A/B working pools on top of the stack: freed before the overlap
        g_ab = ctx.enter_context(ExitStack())
        pha = g_ab.enter_context(tc.tile_pool(name="pha", bufs=2))
        pha_x = g_ab.enter_context(tc.tile_pool(name="pha_x", bufs=3))
        pha_q = g_ab.enter_context(tc.tile_pool(name="pha_q", bufs=1))
        pha_t = g_ab.enter_context(tc.tile_pool(name="pha_t", bufs=2))
        pha_s = g_ab.enter_context(tc.tile_pool(name="pha_s", bufs=4))
        phb = g_ab.enter_context(tc.tile_pool(name="phb", bufs=2))

        st4 = [None]
        w13p = [None]

        def emit_a_ti(ti, _unused):
            if ti % 4 == 0:
                st4[0] = pha.tile([P, 4, D], F16, tag="src_in", name="st4")
                nc.gpsimd.dma_start(
                    st4[0][:], src.ap()[ti * P:(ti + 4) * P, :].rearrange(
                        "(g p) d -> p g d", g=4))
            st = st4[0][:, ti % 4, :]
            sqd = pha_q.tile([P, D], F16, tag="sqd")
            ssq = pha_s.tile([P, 1], F32, tag="ssq")
            nc.vector.scalar_tensor_tensor(
                sqd[:], st, 1.0, st, op0=mybir.AluOpType.mult,
                op1=mybir.AluOpType.mult, accum_out=ssq[:])
            # rms/SX = sqrt(ssq/(D*SX^2) + eps/SX^2) -> rinv = SX/rms
            rms = pha_s.tile([P, 1], F32, tag="rms")
            nc.scalar.activation(rms[:], ssq[:], AF.Sqrt,
                                 bias=eps64_t[:], scale=1.0 / (D * SX * SX))
            rinv = pha_s.tile([P, 1], F32, tag="rinv")
            nc.vector.reciprocal(rinv[:], rms[:])
            xn = pha_x.tile([P, D], F16, tag="xn")
            nc.vector.tensor_scalar_mul(xn[:], st, rinv[:])
            t16 = pha_t.tile([P, DC, P], F16, tag="t16")
            nc.sync.dma_start(t16[:], xn[:], transpose=True)
            nc.scalar.activation(XNT8[:, :, ti * P:(ti + 1) * P], t16[:],
                                 AF.Copy)

        def qk_slice(j, ts, ps_work):
            # DoubleRow qkv projection for 512 tokens of q (j<8) / k
            sl = slice(ts * 512, ts * 512 + 512)
            pk = ps_work.tile([P, 512], F32, tag="work")
            for c2 in range(4):
                nc.tensor.matmul(pk[:], WQ8[:, c2, :, j * P:(j + 1) * P],
                                 XNT8[:, 2 * c2:2 * c2 + 2, sl],
                                 start=(c2 == 0), stop=(c2 == 3),
                                 perf_mode=DR)
            # rope: evict to f16 (true scale), rotate pairs
            pk16 = phb.tile([P, 512], F16, tag="pk16")
            nc.vector.tensor_scalar_mul(pk16[:], pk[:], 1.0 / (SX * SW))
            aa = phb.tile([P, 512], F16, tag="aa")
            nc.vector.tensor_mul(aa[:], pk16[:], C2[:, sl])
            pp = phb.tile([P, 512], F16, tag="pp")
            nc.vector.tensor_mul(pp[:], pk16[:], S2[:, sl])
            bb = phb.tile([P, 512], F16, tag="bb")
            for h0 in (0, 64):
                nc.vector.tensor_copy(bb[h0:h0 + 32, :],
                                      pp[h0 + 32:h0 + 64, :])
                nc.vector.tensor_copy(bb[h0 + 32:h0 + 64, :],
                                      pp[h0:h0 + 32, :])
            dst = (QT[:, j, sl] if j < 8 else KT[:, j - 8, sl])
            nc.vector.tensor_add(dst, aa[:], bb[:])

        def emit_v_ti(ti, ps_work):
            for half in range(2):
                pvh = ps_work.tile([P, 512], F32, tag="work")
                for j4 in range(4):
                    j = 16 + half * 4 + j4
                    for c2 in range(4):
                        nc.tensor.matmul(
                            pvh[:, j4 * P:(j4 + 1) * P],
                            XNT8[:, 2 * c2:2 * c2 + 2, ti * P:(ti + 1) * P],
                            WQ8[:, c2, :, j * P:(j + 1) * P],
                            start=(c2 == 0), stop=(c2 == 3),
                            perf_mode=DR)
                hs = slice(half * 8, (half + 1) * 8)
                nc.vector.memset(va3[:, ti, hs, 64], 1.0)
                nc.vector.tensor_scalar_mul(
                    va3[:, ti, hs, 0:64],
                    pvh.rearrange("p (h c) -> p h c", c=64),
                    SV / (SX * SW))

        def attend(h, qb, ps_st, ps_pv, part=2):
            """Attention for head h, query block qb.  Yields after each
            emission chunk so the caller can interleave other engine work
            (part = number of chunks)."""
            j, hb = h // 2, 64 * (h % 2)
            qsl = slice(qb * QB, (qb + 1) * QB)
            ppv = ps_pv.tile([65, QB], F32, tag="ppv")
            pend = None   # skew: PV for pair kcp emitted after scores kcp+1
            for kcp in range(8):
                pst = ps_st.tile([P, 2, QB], F32, tag="pst")
                for e in range(2):
                    kc = 2 * kcp + e
                    nc.tensor.matmul(
                        pst[:, e, :],
                        KT[hb:hb + 64, j, kc * P:(kc + 1) * P],
                        QT[hb:hb + 64, j, qsl],
                        start=True, stop=True)
                pt8 = phc.tile([P, 2, QB], F8, tag="pt8")
                nc.scalar.activation(pt8[:], pst[:], AF.Exp,
                                     scale=0.125, bias=nbias_t[:])
                if pend is not None:
                    pk, pt = pend
                    nc.tensor.matmul(ppv[:],
                                     VA[:, 2 * pk:2 * pk + 2,
                                        65 * h:65 * h + 65],
                                     pt[:], start=(pk == 0), stop=False,
                                     perf_mode=DR)
                pend = (kcp, pt8)
                if part == 2 and kcp == 3:
                    yield
            pk, pt = pend
            nc.tensor.matmul(ppv[:],
                             VA[:, 2 * pk:2 * pk + 2, 65 * h:65 * h + 65],
                             pt[:], start=False, stop=True, perf_mode=DR)
            rec = phn.tile([1, QB], F32, tag="rec")
            nc.vector.reciprocal(rec[:], ppv[64:65, :])
            nrmS = phn.tile([64, QB], F32, tag="nrmS")
            nc.gpsimd.partition_broadcast(nrmS[:], rec[:])
            nc.vector.tensor_mul(ATT[hb:hb + 64, j, qsl], ppv[0:64, :],
                                 nrmS[:])
            yield

        def run_all(gen):
            for _ in gen:
                pass

        def emit_d_tb(tb, phd_ps, phd_sq):
            py = phd_ps.tile([P, D], F32, tag="py")
            for j in range(8):
                for c2 in range(4):
                    nc.tensor.matmul(py[:, j * P:(j + 1) * P],
                                     ATT[:, 2 * c2:2 * c2 + 2,
                                         tb * P:(tb + 1) * P],
                                     WO8b[:, c2, :, j * P:(j + 1) * P],
                                     start=(c2 == 0), stop=(c2 == 3),
                                     perf_mode=DR)
            srcq = phd.tile([P, D], F16, tag="srcq")
            nc.gpsimd.dma_start(srcq[:], src[tb * P:(tb + 1) * P, :])
            nc.vector.scalar_tensor_tensor(
                X2[:, tb, :], py[:], 1.0 / (SV * SW), srcq[:],
                op0=mybir.AluOpType.mult, op1=mybir.AluOpType.add)
            sq = phd_sq.tile([P, D], F32, tag="sq")
            ssq = phd_s.tile([P, 1], F32, tag="ssq")
            nc.scalar.activation(sq[:], X2[:, tb, :], AF.Square,
                                 accum_out=ssq[:])
            rms = phd_s.tile([P, 1], F32, tag="rms")
            nc.scalar.activation(rms[:], ssq[:], AF.Sqrt,
                                 bias=eps_t[:], scale=1.0 / D)
            rinv = phd_s.tile([P, 1], F32, tag="rinv")
            nc.vector.reciprocal(rinv[:], rms[:])
            xn2 = phd.tile([P, D], F16, tag="xn2")
            nc.vector.tensor_scalar_mul(xn2[:], X2[:, tb, :], rinv[:])
            nc.sync.dma_start(XN2T[:, :, tb * P:(tb + 1) * P], xn2[:],
                              transpose=True)

        def emit_e_fc(fc, qb, phe_ps, dge=None):
            qsl = slice(qb * QB, (qb + 1) * QB)
            w13c = phe_w.tile([P, DC, 2, P], F16, tag="w13c")
            (dge or nc.sync).dma_start(w13c[:],
                                       w13t[:, :, :, fc * P:(fc + 1) * P])
            pab = phe_ps.tile([P, 2, QB], F32, tag="pab")
            for s in range(2):
                for c in range(DC):
                    nc.tensor.matmul(pab[:, s, :], w13c[:, c, s, :],
                                     XN2T[:, c, qsl],
                                     start=(c == 0), stop=(c == DC - 1))
            # silu via tanh (same ACT table as Exp -> no table reloads):
            # 2*silu(x) = x*(1+tanh(x/2)); the 1/2 is folded into w3t.
            th = phe.tile([P, QB], F16, tag="th")
            nc.scalar.activation(th[:], pab[:, 0, :], AF.Tanh, scale=0.5)
            sa = phe.tile([P, QB], F16, tag="sa")
            nc.vector.scalar_tensor_tensor(
                sa[:], th[:], 1.0, pab[:, 0, :],
                op0=mybir.AluOpType.add, op1=mybir.AluOpType.mult)
            # HT8 holds one query block at a time (reused across qb passes)
            nc.vector.tensor_mul(HT8[:, fc, :], sa[:], pab[:, 1, :])

        def emit_e_fc_h(fc, phe_ps):
            # E for query block 0 in two 256-wide passes (1-bank PSUM tiles)
            for half in range(2):
                qsl = slice(half * 256, half * 256 + 256)
                w13c = phe_w.tile([P, DC, 2, P], F16, tag="w13c")
                if half == 0:
                    nc.sync.dma_start(w13c[:],
                                      w13t[:, :, :, fc * P:(fc + 1) * P])
                    w13k = w13c
                else:
                    w13k = w13p[0]
                w13p[0] = w13k
                pab = phe_ps.tile([P, 2, 256], F32, tag="pab")
                for s in range(2):
                    for c in range(DC):
                        nc.tensor.matmul(pab[:, s, :], w13k[:, c, s, :],
                                         XN2T[:, c, qsl],
                                         start=(c == 0), stop=(c == DC - 1))
                th = phe.tile([P, 256], F16, tag="th")
                nc.scalar.activation(th[:], pab[:, 0, :], AF.Tanh, scale=0.5)
                sa = phe.tile([P, 256], F16, tag="sa")
                nc.vector.scalar_tensor_tensor(
                    sa[:], th[:], 1.0, pab[:, 0, :],
                    op0=mybir.AluOpType.add, op1=mybir.AluOpType.mult)
                nc.vector.tensor_mul(HT8[:, fc, qsl], sa[:], pab[:, 1, :])

        def emit_f_j(j, qb, phf_ps, dge=None):
            qsl = slice(0, QB)
            w2j = phf_w.tile([P, FC // 2, 2, P], F8, tag="w2j")
            (dge or nc.sync).dma_start(w2j[:], w28[:, :, :, j * P:(j + 1) * P])
            pz = phf_ps.tile([P, QB], F32, tag="pz")
            for fp in range(FC // 2):
                nc.tensor.matmul(pz[:],
                                 w2j[:, fp, :, :],
                                 HT8[:, 2 * fp:2 * fp + 2, qsl],
                                 start=(fp == 0), stop=(fp == FC // 2 - 1),
                                 perf_mode=DR)
            yj = phf.tile([P, QB], F16, tag="yj")
            nc.vector.tensor_scalar_mul(yj[:], pz[:], 1.0 / (SH * SW))
            yT = phf.tile([P, QB // P, P], F16, tag="yT")
            nc.sync.dma_start(yT[:], yj[:], transpose=True)
            for tq in range(QB // P):
                tb = qb * (QB // P) + tq
                nc.vector.tensor_add(X2[:, tb, j * P:(j + 1) * P],
                                     X2[:, tb, j * P:(j + 1) * P],
                                     yT[:, tq, :])

        # ---------------- A + B + C(qb0), woven ----------------
        with tc.tile_pool(name="ps_work", bufs=2, space="PSUM") as ps_work, \
             tc.tile_pool(name="ps_st", bufs=2, space="PSUM") as ps_st, \
             tc.tile_pool(name="ps_pv", bufs=2, space="PSUM") as ps_pv:
            for ti in range(4):
                emit_a_ti(ti, None)
            emit_v_ti(0, ps_work)
            qk_slice(0, 0, ps_work)
            for ti in range(4, 8):
                emit_a_ti(ti, None)
                emit_v_ti(ti - 3, ps_work)
            qk_slice(0, 1, ps_work)
            qk_slice(8, 0, ps_work)
            for ti in range(8, 12):
                emit_a_ti(ti, None)
                emit_v_ti(ti - 3, ps_work)
            qk_slice(8, 1, ps_work)
            qk_slice(8, 2, ps_work)
            for ti in range(12, 16):
                emit_a_ti(ti, None)
                emit_v_ti(ti - 3, ps_work)
            qk_slice(8, 3, ps_work)
            for ti in range(13, S // P):
                emit_v_ti(ti, ps_work)
            run_all(attend(0, 0, ps_st, ps_pv))
            run_all(attend(1, 0, ps_st, ps_pv))
            for hp in range(1, 8):
                for jj in (hp, 8 + hp):
                    for ts in range(2 if jj < 8 else 4):
                        qk_slice(jj, ts, ps_work)
                run_all(attend(2 * hp, 0, ps_st, ps_pv))
                run_all(attend(2 * hp + 1, 0, ps_st, ps_pv))
        g_xnt.close()   # free XNT8 / WQ8 / C2 / S2
        g_ab.close()    # free A/B working pools

        # D/E/F big tiles go where XNT8/WQ8 were
        p_mid = ctx.enter_context(tc.tile_pool(name="p_mid", bufs=1,
                                               side="right"))
        X2 = p_mid.tile([P, TQ // P, D], F32)    # residual+output (t-major)
        XN2T = p_mid.tile([P, DC, TQ], F16)      # x_norm2.T
        HT8 = p_mid.tile([P, FC, QB], F8)        # swiglu hidden .T (x8)

        # ---------------- D(qb0) ----------------
        with tc.tile_pool(name="phd_ps", bufs=2, space="PSUM") as phd_ps, \
             tc.tile_pool(name="phd_sq", bufs=2, space="PSUM") as phd_sq:
            for tb in range(4):
                emit_d_tb(tb, phd_ps, phd_sq)

        # ---------- overlap: C(qb1) interleaved with E(qb0) ----------
        with tc.tile_pool(name="ps_st2", bufs=2, space="PSUM") as ps_st2, \
             tc.tile_pool(name="ps_pv2", bufs=2, space="PSUM") as ps_pv2, \
             tc.tile_pool(name="phe_ps", bufs=1, space="PSUM") as phe_ps:
            for h in range(NH):
                gen = attend(h, 1, ps_st2, ps_pv2)
                next(gen)
                emit_e_fc(2 * h, 0, phe_ps)
                run_all(gen)
                emit_e_fc(2 * h + 1, 0, phe_ps)
        # p_qkv (QT/KT/VA) stays allocated; fits alongside p_mid

        # ---------------- D(qb1), then F(qb0) hiding D1's latency ------
        with tc.tile_pool(name="phd_ps", bufs=2, space="PSUM") as phd_ps, \
             tc.tile_pool(name="phd_sq", bufs=2, space="PSUM") as phd_sq:
            for tb in range(4, 8):
                emit_d_tb(tb, phd_ps, phd_sq)
        with tc.tile_pool(name="phf_ps", bufs=2, space="PSUM") as phf_ps:
            for j in range(8):
                emit_f_j(j, 0, phf_ps)
            for tb in range(4):
                nc.sync.dma_start(out[tb * P:(tb + 1) * P, :], X2[:, tb, :])

        # ---------------- E(qb1) + F(qb1) ----------------
        with tc.tile_pool(name="phe_ps2", bufs=3, space="PSUM") as phe_ps2:
            for fc in range(FC):
                emit_e_fc(fc, 1, phe_ps2)
        with tc.tile_pool(name="phf_ps", bufs=2, space="PSUM") as phf_ps:
            for j in range(8):
                emit_f_j(j, 1, phf_ps)
            for tb in range(4, 8):
                nc.sync.dma_start(out[tb * P:(tb + 1) * P, :], X2[:, tb, :])


_NC_CACHE = None


def _get_nc():
    global _NC_CACHE
    if _NC_CACHE is None:
        _NC_CACHE = build_nc()
    return _NC_CACHE


def _host_tables(positions_b, axis_scale):
    """Build parity-split fp16 cos/sin tables (128, S) for one batch."""
    coord = positions_b * axis_scale[None, :]              # (S, 4)
    invf = BASE ** (-(np.arange(0, 16, 2, dtype=np.float32) / 16.0))  # (8,)
    ang = coord[:, :, None] * invf[None, None, :]          # (S, 4, 8)
    ang = ang.reshape(S, 32).T                             # (32, S): r = 8a + j
    cos64 = np.concatenate([np.cos(ang), np.cos(ang)], axis=0)   # (64, S)
    sin64 = np.concatenate([np.sin(ang), -np.sin(ang)], axis=0)  # (64, S)
    c128 = np.concatenate([cos64, cos64], axis=0).astype(np.float16)
    s128 = np.concatenate([sin64, sin64], axis=0).astype(np.float16)
    return c128, s128


def _prep_weights(inputs):
    """Pre-transpose / permute / scale / quantize all weights on the host."""
    n1 = np.asarray(inputs["norm1_w"], np.float32)
    n2 = np.asarray(inputs["norm2_w"], np.float32)
    w_qkv = np.asarray(inputs["w_qkv"], np.float32) * n1[None, :]
    w_out = np.asarray(inputs["w_out"], np.float32)
    w1 = np.asarray(inputs["w1"], np.float32) * n2[None, :]
    w3 = np.asarray(inputs["w3"], np.float32) * n2[None, :]
    w2 = np.asarray(inputs["w2"], np.float32)

    # qkv row permutation: for j<16 (q,k) parity split rows; v plain
    rows = np.zeros(3 * D, np.int64)
    for j in range(24):
        for col in range(P):
            if j < 16:
                h, r = divmod(col, 64)
                par, jp = divmod(r, 32)
                rows[j * P + col] = 128 * j + 64 * h + 2 * jp + par
            else:
                rows[j * P + col] = j * P + col
    wq_perm = w_qkv[rows, :]                   # (3D, D) row-permuted
    # wqkv8[p, c2, e, jf]  = wq_perm[jf, (2*c2+e)*128 + p] * SW
    wqkv8 = (wq_perm.T.reshape(4, 2, P, 3 * D).transpose(2, 0, 1, 3)
             * SW).astype(NP8)
    wqkv8 = np.ascontiguousarray(wqkv8)
    # wo8[p, c2, e, f] = w_out[f, (2*c2+e)*128 + p] * SW
    wo8 = (w_out.T.reshape(4, 2, P, D).transpose(2, 0, 1, 3) * SW).astype(NP8)
    wo8 = np.ascontiguousarray(wo8)
    # w13t[p, c, s, f]: s=0 -> w1[f, c*128+p], s=1 -> w3[f, c*128+p]*SH/2
    w1tt = w1.T.reshape(DC, P, DFF).transpose(1, 0, 2)
    w3tt = w3.T.reshape(DC, P, DFF).transpose(1, 0, 2) * (SH / 2)
    w13t = np.ascontiguousarray(
        np.stack([w1tt, w3tt], axis=2)).astype(np.float16)
    # w28[p, fp, e, f] = w2[f, (2*fp+e)*128 + p] * SW
    w28 = (w2.T.reshape(FC // 2, 2, P, D).transpose(2, 0, 1, 3)
           * SW).astype(NP8)
    w28 = np.ascontiguousarray(w28)
    return {"wqkv8": wqkv8, "wo8": wo8, "w13t": w13t, "w28": w28}


def build_in_maps(inputs):
    src = np.asarray(inputs["src"], dtype=np.float32)
    positions = np.asarray(inputs["positions"], dtype=np.float32)
    axis_scale = np.asarray(inputs["axis_scale"], np.float32)
    weights = _prep_weights(inputs)
    in_maps = []
    for c in range(N_CORES):
        b, h = c // 2, c % 2
        sp = src[b]
        pp = positions[b]
        if h == 1:  # own half first
            sp = np.concatenate([sp[TQ:], sp[:TQ]], axis=0)
            pp = np.concatenate([pp[TQ:], pp[:TQ]], axis=0)
        ct, st = _host_tables(pp, axis_scale)
        m = {"src": np.ascontiguousarray(sp), "cos_t": ct, "sin_t": st}
        m.update(weights)
        in_maps.append(m)
    return in_maps


def kernel(src, positions, w_qkv, w_out, norm1_w, norm2_w, w1, w2, w3,
           axis_scale):
    src = np.asarray(src, dtype=np.float32)
    B = src.shape[0]
    in_maps = build_in_maps(dict(
        src=src, positions=positions, w_qkv=w_qkv, w_out=w_out,
        norm1_w=norm1_w, norm2_w=norm2_w, w1=w1, w2=w2, w3=w3,
        axis_scale=axis_scale))
    nc = _get_nc()
    res = bass_utils.run_bass_kernel_spmd(nc, in_maps,
                                          core_ids=list(range(N_CORES)))
    outp = np.zeros((B, S, D), np.float32)
    for c in range(N_CORES):
        b, h = c // 2, c % 2
        outp[b, h * TQ:(h + 1) * TQ, :] = res.results[c]["out"]
    return outp


# revision 4
# speedup vs baseline: 1.0904x; 1.0598x over previous
"""Trainium2 Bass kernel: NeptuneTransformerEncoderLayer on 8 NeuronCores.

Sharding: batch(4) x seq-half(2) -> 8 cores, zero collectives.  Each core
computes K/V for its batch's full 2048 tokens and Q/attention/FFN for its
own 1024 tokens (host permutes src so own queries are rows [0:1024)).

v2: fp8e4 DoubleRow matmuls (0.5 cyc/row) for QKV, PV, out-proj and FFN
down-proj; f16 for scores and FFN up-proj (precision).  Weights are
host-side pre-transposed/pre-scaled/pre-quantized (no on-device weight
transposes).  The ACT-bound attention is split into two 512-query blocks
and the second block's attention is interleaved with the first block's
PE-bound FFN so both engines stay busy.  Scores PSUM tiles pair kc chunks
so exp keeps 1024-wide slices; PV emission is skewed one kc-pair behind
scores so the PE never waits on exp.
"""
import sys

for _p in ("/opt/trn_rl_repo", "/root/.axon_site/_ro/trn_rl_repo"):
    if _p not in sys.path:
        sys.path.insert(0, _p)

import numpy as np
import ml_dtypes

import concourse.bass as bass
import concourse.mybir as mybir
import concourse.tile as tile
from concourse import bacc
from concourse import bass_utils

F8 = mybir.dt.float8e4
F16 = mybir.dt.float16
F32 = mybir.dt.float32
AF = mybir.ActivationFunctionType
DR = mybir.MatmulPerfMode.DoubleRow
NP8 = ml_dtypes.float8_e4m3

P = 128
D = 1024            # d_model
DC = D // P         # 8 d-model chunks
NH = 16             # heads
HD = 64             # head dim
DFF = 4096
FC = DFF // P       # 32 ff chunks
S = 2048            # full sequence per batch
TQ = 1024           # query tokens per core
QB = 512            # query pipeline block
N_CORES = 8
EPS = 1e-5
BASE = 10000.0

SX = 8.0            # x_norm fp8 scale
SW = 64.0           # fp8 weight scale
SV = 8.0            # v fp8 scale
SH = 8.0            # hidden fp8 scale (via w3)
EXP_BIAS = 1.0      # exp(0.125*s - EXP_BIAS); score stats: max 4.36, rowmax>=1.2


def build_nc():
    nc = bacc.Bacc("TRN2", target_bir_lowering=False, debug=False,
                   num_devices=N_CORES)
    src = nc.dram_tensor("src", [S, D], F32, kind="ExternalInput")
    cos_t = nc.dram_tensor("cos_t", [P, S], F16, kind="ExternalInput")
    sin_t = nc.dram_tensor("sin_t", [P, S], F16, kind="ExternalInput")
    wqkv8 = nc.dram_tensor("wqkv8", [P, 4, 2, 3 * D], F8, kind="ExternalInput")
    wo8 = nc.dram_tensor("wo8", [P, 4, 2, D], F8, kind="ExternalInput")
    w13t = nc.dram_tensor("w13t", [P, DC, 2, DFF], F16, kind="ExternalInput")
    w28 = nc.dram_tensor("w28", [P, FC // 2, 2, D], F8, kind="ExternalInput")
    out = nc.dram_tensor("out", [TQ, D], F32, kind="ExternalOutput")

    with tile.TileContext(nc) as tc:
        emit(nc, tc, src, cos_t, sin_t, wqkv8, wo8, w13t, w28, out)
    nc.compile()
    return nc


def emit(nc, tc, src, cos_t, sin_t, wqkv8, wo8, w13t, w28, out):
    from contextlib import ExitStack

    ctx = ExitStack()
    with ctx:
        g_xnt = ctx.enter_context(ExitStack())   # XNT8/C2/S2/WQ8, freed post-B
        persist = ctx.enter_context(tc.tile_pool(name="persist", bufs=1))
        p_att = ctx.enter_context(tc.tile_pool(name="p_att", bufs=1))
        p_xnt = g_xnt.enter_context(
            tc.tile_pool(name="p_xnt", bufs=1, side="right"))
        p_qkv = ctx.enter_context(tc.tile_pool(name="p_qkv", bufs=1))

        XNT8 = p_xnt.tile([P, DC, S], F8)        # x_norm1.T fp8 (x8)
        WQ8 = p_xnt.tile([P, 4, 2, 3 * D], F8)   # qkv weights (24KB/p)
        C2 = p_xnt.tile([P, S], F16)
        S2 = p_xnt.tile([P, S], F16)
        QT = p_qkv.tile([P, DC, TQ], F16)        # roped q.T (true scale)
        KT = p_qkv.tile([P, DC, S], F16)         # roped k.T
        VA = p_qkv.tile([P, S // P, NH * 65], F8)   # v8 + ones col per head
        ATT = p_att.tile([P, DC, TQ], F8)        # attn out .T, fp8 (x8)
        WO8b = persist.tile([P, 4, 2, D], F8)
        eps_t = persist.tile([P, 1], F32)
        nc.vector.memset(eps_t[:], EPS)
        eps64_t = persist.tile([P, 1], F32)
        nc.vector.memset(eps64_t[:], EPS / (SX * SX))
        nbias_t = persist.tile([P, 1], F32)
        nc.vector.memset(nbias_t[:], -EXP_BIAS)

        nc.sync.dma_start(C2[:], cos_t[:])
        nc.sync.dma_start(S2[:], sin_t[:])
        nc.scalar.dma_start(WQ8[:], wqkv8[:])
        nc.scalar.dma_start(WO8b[:], wo8[:])

        va3 = VA.rearrange("p t (h c) -> p t h c", c=65)

        # long-lived working pools first (LIFO stack: closed last)
        phc = ctx.enter_context(tc.tile_pool(name="phc", bufs=2))
        phn = ctx.enter_context(tc.tile_pool(name="phn", bufs=1))
        pha_q = ctx.enter_context(tc.tile_pool(name="pha_q", bufs=1))
        phd = ctx.enter_context(tc.tile_pool(name="phd", bufs=2))
        phd_s = ctx.enter_context(tc.tile_pool(name="phd_s", bufs=4))
        phe_w = ctx.enter_context(tc.tile_pool(name="phe_w", bufs=3))
        phe = ctx.enter_context(tc.tile_pool(name="phe", bufs=2))
        phf = ctx.enter_context(tc.tile_pool(name="phf", bufs=2))
        phf_w = ctx.enter_context(tc.tile_pool(name="phf_w", bufs=2))
        # A/B working pools on top of the stack: freed before the overlap
        g_ab = ctx.enter_context(ExitStack())
        pha = g_ab.enter_context(tc.tile_pool(name="pha", bufs=2))
        pha_x = g_ab.enter_context(tc.tile_pool(name="pha_x", bufs=3))
        pha_t = g_ab.enter_context(tc.tile_pool(name="pha_t", bufs=2))
        pha_s = g_ab.enter_context(tc.tile_pool(name="pha_s", bufs=4))
        phb = g_ab.enter_context(tc.tile_pool(name="phb", bufs=2))

        st4 = [None]
        w13p = [None]

        def emit_a_ti(ti, _unused):
            if ti % 4 == 0:
                st4[0] = pha.tile([P, 4, D], F16, tag="src_in", name="st4")
                nc.gpsimd.dma_start(
                    st4[0][:], src.ap()[ti * P:(ti + 4) * P, :].rearrange(
                        "(g p) d -> p g d", g=4))
            st = st4[0][:, ti % 4, :]
            sqd = pha_q.tile([P, D], F16, tag="sqd")
            ssq = pha_s.tile([P, 1], F32, tag="ssq")
            nc.vector.scalar_tensor_tensor(
                sqd[:], st, 1.0, st, op0=mybir.AluOpType.mult,
                op1=mybir.AluOpType.mult, accum_out=ssq[:])
            # rms/SX = sqrt(ssq/(D*SX^2) + eps/SX^2) -> rinv = SX/rms
            rms = pha_s.tile([P, 1], F32, tag="rms")
            nc.scalar.activation(rms[:], ssq[:], AF.Sqrt,
                                 bias=eps64_t[:], scale=1.0 / (D * SX * SX))
            rinv = pha_s.tile([P, 1], F32, tag="rinv")
            nc.vector.reciprocal(rinv[:], rms[:])
            xn = pha_x.tile([P, D], F16, tag="xn")
            nc.vector.tensor_scalar_mul(xn[:], st, rinv[:])
            t16 = pha_t.tile([P, DC, P], F16, tag="t16")
            nc.sync.dma_start(t16[:], xn[:], transpose=True)
            nc.scalar.activation(XNT8[:, :, ti * P:(ti + 1) * P], t16[:],
                                 AF.Copy)

        def qk_slice(j, ts, ps_work):
            # DoubleRow qkv projection for 512 tokens of q (j<8) / k
            sl = slice(ts * 512, ts * 512 + 512)
            pk = ps_work.tile([P, 512], F32, tag="work")
            for c2 in range(4):
                nc.tensor.matmul(pk[:], WQ8[:, c2, :, j * P:(j + 1) * P],
                                 XNT8[:, 2 * c2:2 * c2 + 2, sl],
                                 start=(c2 == 0), stop=(c2 == 3),
                                 perf_mode=DR)
            # rope: evict to f16 (true scale), rotate pairs
            pk16 = phb.tile([P, 512], F16, tag="pk16")
            nc.vector.tensor_scalar_mul(pk16[:], pk[:], 1.0 / (SX * SW))
            aa = phb.tile([P, 512], F16, tag="aa")
            nc.vector.tensor_mul(aa[:], pk16[:], C2[:, sl])
            pp = phb.tile([P, 512], F16, tag="pp")
            nc.vector.tensor_mul(pp[:], pk16[:], S2[:, sl])
            bb = phb.tile([P, 512], F16, tag="bb")
            for h0 in (0, 64):
                nc.vector.tensor_copy(bb[h0:h0 + 32, :],
                                      pp[h0 + 32:h0 + 64, :])
                nc.vector.tensor_copy(bb[h0 + 32:h0 + 64, :],
                                      pp[h0:h0 + 32, :])
            dst = (QT[:, j, sl] if j < 8 else KT[:, j - 8, sl])
            nc.vector.tensor_add(dst, aa[:], bb[:])

        def emit_v_ti(ti, ps_work):
            for half in range(2):
                pvh = ps_work.tile([P, 512], F32, tag="work")
                for j4 in range(4):
                    j = 16 + half * 4 + j4
                    for c2 in range(4):
                        nc.tensor.matmul(
                            pvh[:, j4 * P:(j4 + 1) * P],
                            XNT8[:, 2 * c2:2 * c2 + 2, ti * P:(ti + 1) * P],
                            WQ8[:, c2, :, j * P:(j + 1) * P],
                            start=(c2 == 0), stop=(c2 == 3),
                            perf_mode=DR)
                hs = slice(half * 8, (half + 1) * 8)
                nc.vector.memset(va3[:, ti, hs, 64], 1.0)
                nc.vector.tensor_scalar_mul(
                    va3[:, ti, hs, 0:64],
                    pvh.rearrange("p (h c) -> p h c", c=64),
                    SV / (SX * SW))

        def attend(h, qb, ps_st, ps_pv, part=2):
            """Attention for head h, query block qb.  Yields after each
            emission chunk so the caller can interleave other engine work
            (part = number of chunks)."""
            j, hb = h // 2, 64 * (h % 2)
            qsl = slice(qb * QB, (qb + 1) * QB)
            ppv = ps_pv.tile([65, QB], F32, tag="ppv")
            pend = None   # skew: PV for pair kcp emitted after scores kcp+1
            for kcp in range(8):
                pst = ps_st.tile([P, 2, QB], F32, tag="pst")
                for e in range(2):
                    kc = 2 * kcp + e
                    nc.tensor.matmul(
                        pst[:, e, :],
                        KT[hb:hb + 64, j, kc * P:(kc + 1) * P],
                        QT[hb:hb + 64, j, qsl],
                        start=True, stop=True)
                pt8 = phc.tile([P, 2, QB], F8, tag="pt8")
                nc.scalar.activation(pt8[:], pst[:], AF.Exp,
                                     scale=0.125, bias=nbias_t[:])
                if pend is not None:
                    pk, pt = pend
                    nc.tensor.matmul(ppv[:],
                                     VA[:, 2 * pk:2 * pk + 2,
                                        65 * h:65 * h + 65],
                                     pt[:], start=(pk == 0), stop=False,
                                     perf_mode=DR)
                pend = (kcp, pt8)
                if part == 2 and kcp == 3:
                    yield
            pk, pt = pend
            nc.tensor.matmul(ppv[:],
                             VA[:, 2 * pk:2 * pk + 2, 65 * h:65 * h + 65],
                             pt[:], start=False, stop=True, perf_mode=DR)
            rec = phn.tile([1, QB], F32, tag="rec")
            nc.vector.reciprocal(rec[:], ppv[64:65, :])
            nrmS = phn.tile([64, QB], F32, tag="nrmS")
            nc.gpsimd.partition_broadcast(nrmS[:], rec[:])
            nc.vector.tensor_mul(ATT[hb:hb + 64, j, qsl], ppv[0:64, :],
                                 nrmS[:])
            yield

        def run_all(gen):
            for _ in gen:
                pass

        def emit_d_tb(tb, phd_ps, _unused):
            py = phd_ps.tile([P, D], F32, tag="py")
            for j in range(8):
                for c2 in range(4):
                    nc.tensor.matmul(py[:, j * P:(j + 1) * P],
                                     ATT[:, 2 * c2:2 * c2 + 2,
                                         tb * P:(tb + 1) * P],
                                     WO8b[:, c2, :, j * P:(j + 1) * P],
                                     start=(c2 == 0), stop=(c2 == 3),
                                     perf_mode=DR)
            srcq = phd.tile([P, D], F16, tag="srcq")
            nc.gpsimd.dma_start(srcq[:], src[tb * P:(tb + 1) * P, :])
            nc.vector.scalar_tensor_tensor(
                X2[:, tb, :], py[:], 1.0 / (SV * SW), srcq[:],
                op0=mybir.AluOpType.mult, op1=mybir.AluOpType.add)
            sqd2 = pha_q.tile([P, D], F16, tag="sqd")
            ssq = phd_s.tile([P, 1], F32, tag="ssq")
            nc.vector.scalar_tensor_tensor(
                sqd2[:], X2[:, tb, :], 1.0, X2[:, tb, :],
                op0=mybir.AluOpType.mult, op1=mybir.AluOpType.mult,
                accum_out=ssq[:])
            rms = phd_s.tile([P, 1], F32, tag="rms")
            nc.scalar.activation(rms[:], ssq[:], AF.Sqrt,
                                 bias=eps_t[:], scale=1.0 / D)
            rinv = phd_s.tile([P, 1], F32, tag="rinv")
            nc.vector.reciprocal(rinv[:], rms[:])
            xn2 = phd.tile([P, D], F16, tag="xn2")
            nc.vector.tensor_scalar_mul(xn2[:], X2[:, tb, :], rinv[:])
            nc.sync.dma_start(XN2T[:, :, tb * P:(tb + 1) * P], xn2[:],
                              transpose=True)

        def emit_e_fc(fc, qb, phe_ps, dge=None):
            qsl = slice(qb * QB, (qb + 1) * QB)
            w13c = phe_w.tile([P, DC, 2, P], F16, tag="w13c")
            (dge or nc.sync).dma_start(w13c[:],
                                       w13t[:, :, :, fc * P:(fc + 1) * P])
            pab = phe_ps.tile([P, 2, QB], F32, tag="pab")
            for s in range(2):
                for c in range(DC):
                    nc.tensor.matmul(pab[:, s, :], w13c[:, c, s, :],
                                     XN2T[:, c, qsl],
                                     start=(c == 0), stop=(c == DC - 1))
            # silu via tanh (same ACT table as Exp -> no table reloads):
            # 2*silu(x) = x*(1+tanh(x/2)); the 1/2 is folded into w3t.
            th = phe.tile([P, QB], F16, tag="th")
            nc.scalar.activation(th[:], pab[:, 0, :], AF.Tanh, scale=0.5)
            sa = phe.tile([P, QB], F16, tag="sa")
            nc.vector.scalar_tensor_tensor(
                sa[:], th[:], 1.0, pab[:, 0, :],
                op0=mybir.AluOpType.add, op1=mybir.AluOpType.mult)
            # HT8 holds one query block at a time (reused across qb passes)
            nc.vector.tensor_mul(HT8[:, fc, :], sa[:], pab[:, 1, :])

        def emit_e_fc_h(fc, phe_ps):
            # E for query block 0 in two 256-wide passes (1-bank PSUM tiles)
            for half in range(2):
                qsl = slice(half * 256, half * 256 + 256)
                w13c = phe_w.tile([P, DC, 2, P], F16, tag="w13c")
                if half == 0:
                    nc.sync.dma_start(w13c[:],
                                      w13t[:, :, :, fc * P:(fc + 1) * P])
                    w13k = w13c
                else:
                    w13k = w13p[0]
                w13p[0] = w13k
                pab = phe_ps.tile([P, 2, 256], F32, tag="pab")
                for s in range(2):
                    for c in range(DC):
                        nc.tensor.matmul(pab[:, s, :], w13k[:, c, s, :],
                                         XN2T[:, c, qsl],
                                         start=(c == 0), stop=(c == DC - 1))
                th = phe.tile([P, 256], F16, tag="th")
                nc.scalar.activation(th[:], pab[:, 0, :], AF.Tanh, scale=0.5)
                sa = phe.tile([P, 256], F16, tag="sa")
                nc.vector.scalar_tensor_tensor(
                    sa[:], th[:], 1.0, pab[:, 0, :],
                    op0=mybir.AluOpType.add, op1=mybir.AluOpType.mult)
                nc.vector.tensor_mul(HT8[:, fc, qsl], sa[:], pab[:, 1, :])

        def emit_f_j(j, qb, phf_ps, dge=None):
            qsl = slice(0, QB)
            w2j = phf_w.tile([P, FC // 2, 2, P], F8, tag="w2j")
            (dge or nc.sync).dma_start(w2j[:], w28[:, :, :, j * P:(j + 1) * P])
            pz = phf_ps.tile([P, QB], F32, tag="pz")
            for fp in range(FC // 2):
                nc.tensor.matmul(pz[:],
                                 w2j[:, fp, :, :],
                                 HT8[:, 2 * fp:2 * fp + 2, qsl],
                                 start=(fp == 0), stop=(fp == FC // 2 - 1),
                                 perf_mode=DR)
            yj = phf.tile([P, QB], F16, tag="yj")
            nc.vector.tensor_scalar_mul(yj[:], pz[:], 1.0 / (SH * SW))
            yT = phf.tile([P, QB // P, P], F16, tag="yT")
            nc.sync.dma_start(yT[:], yj[:], transpose=True)
            for tq in range(QB // P):
                tb = qb * (QB // P) + tq
                nc.vector.tensor_add(X2[:, tb, j * P:(j + 1) * P],
                                     X2[:, tb, j * P:(j + 1) * P],
                                     yT[:, tq, :])

        # ---------------- A + B + C(qb0), woven ----------------
        with tc.tile_pool(name="ps_work", bufs=2, space="PSUM") as ps_work, \
             tc.tile_pool(name="ps_st", bufs=2, space="PSUM") as ps_st, \
             tc.tile_pool(name="ps_pv", bufs=2, space="PSUM") as ps_pv:
            for ti in range(4):
                emit_a_ti(ti, None)
            emit_v_ti(0, ps_work)
            qk_slice(0, 0, ps_work)
            for ti in range(4, 8):
                emit_a_ti(ti, None)
                emit_v_ti(ti - 3, ps_work)
            qk_slice(0, 1, ps_work)
            qk_slice(8, 0, ps_work)
            for ti in range(8, 12):
                emit_a_ti(ti, None)
                emit_v_ti(ti - 3, ps_work)
            qk_slice(8, 1, ps_work)
            qk_slice(8, 2, ps_work)
            for ti in range(12, 16):
                emit_a_ti(ti, None)
                emit_v_ti(ti - 3, ps_work)
            qk_slice(8, 3, ps_work)
            for ti in range(13, S // P):
                emit_v_ti(ti, ps_work)
            run_all(attend(0, 0, ps_st, ps_pv))
            run_all(attend(1, 0, ps_st, ps_pv))
            for hp in range(1, 8):
                for jj in (hp, 8 + hp):
                    for ts in range(2 if jj < 8 else 4):
                        qk_slice(jj, ts, ps_work)
                run_all(attend(2 * hp, 0, ps_st, ps_pv))
                run_all(attend(2 * hp + 1, 0, ps_st, ps_pv))
        g_xnt.close()   # free XNT8 / WQ8 / C2 / S2
        g_ab.close()    # free A/B working pools

        # D/E/F big tiles go where XNT8/WQ8 were
        p_mid = ctx.enter_context(tc.tile_pool(name="p_mid", bufs=1,
                                               side="right"))
        X2 = p_mid.tile([P, TQ // P, D], F32)    # residual+output (t-major)
        XN2T = p_mid.tile([P, DC, TQ], F16)      # x_norm2.T
        HT8 = p_mid.tile([P, FC, QB], F8)        # swiglu hidden .T (x8)

        # ---------------- D(qb0) ----------------
        with tc.tile_pool(name="phd_ps", bufs=4, space="PSUM") as phd_ps:
            for tb in range(4):
                emit_d_tb(tb, phd_ps, None)

        # ---------- overlap: C(qb1) interleaved with E(qb0) ----------
        with tc.tile_pool(name="ps_st2", bufs=2, space="PSUM") as ps_st2, \
             tc.tile_pool(name="ps_pv2", bufs=2, space="PSUM") as ps_pv2, \
             tc.tile_pool(name="phe_ps", bufs=1, space="PSUM") as phe_ps:
            for h in range(NH):
                gen = attend(h, 1, ps_st2, ps_pv2)
                next(gen)
                emit_e_fc(2 * h, 0, phe_ps)
                run_all(gen)
                emit_e_fc(2 * h + 1, 0, phe_ps)
        # p_qkv (QT/KT/VA) stays allocated; fits alongside p_mid

        # ---------------- D(qb1), then F(qb0) hiding D1's latency ------
        with tc.tile_pool(name="phd_ps", bufs=4, space="PSUM") as phd_ps:
            for tb in range(4, 8):
                emit_d_tb(tb, phd_ps, None)
        with tc.tile_pool(name="phf_ps", bufs=2, space="PSUM") as phf_ps:
            for j in range(8):
                emit_f_j(j, 0, phf_ps)
            for tb in range(4):
                nc.sync.dma_start(out[tb * P:(tb + 1) * P, :], X2[:, tb, :])

        # ---------------- E(qb1) + F(qb1) ----------------
        with tc.tile_pool(name="phe_ps2", bufs=3, space="PSUM") as phe_ps2:
            for fc in range(FC):
                emit_e_fc(fc, 1, phe_ps2)
        with tc.tile_pool(name="phf_ps", bufs=2, space="PSUM") as phf_ps:
            for j in range(8):
                emit_f_j(j, 1, phf_ps)
            for tb in range(4, 8):
                nc.sync.dma_start(out[tb * P:(tb + 1) * P, :], X2[:, tb, :])


_NC_CACHE = None


def _get_nc():
    global _NC_CACHE
    if _NC_CACHE is None:
        _NC_CACHE = build_nc()
    return _NC_CACHE


def _host_tables(positions_b, axis_scale):
    """Build parity-split fp16 cos/sin tables (128, S) for one batch."""
    coord = positions_b * axis_scale[None, :]              # (S, 4)
    invf = BASE ** (-(np.arange(0, 16, 2, dtype=np.float32) / 16.0))  # (8,)
    ang = coord[:, :, None] * invf[None, None, :]          # (S, 4, 8)
    ang = ang.reshape(S, 32).T                             # (32, S): r = 8a + j
    cos64 = np.concatenate([np.cos(ang), np.cos(ang)], axis=0)   # (64, S)
    sin64 = np.concatenate([np.sin(ang), -np.sin(ang)], axis=0)  # (64, S)
    c128 = np.concatenate([cos64, cos64], axis=0).astype(np.float16)
    s128 = np.concatenate([sin64, sin64], axis=0).astype(np.float16)
    return c128, s128


def _prep_weights(inputs):
    """Pre-transpose / permute / scale / quantize all weights on the host."""
    n1 = np.asarray(inputs["norm1_w"], np.float32)
    n2 = np.asarray(inputs["norm2_w"], np.float32)
    w_qkv = np.asarray(inputs["w_qkv"], np.float32) * n1[None, :]
    w_out = np.asarray(inputs["w_out"], np.float32)
    w1 = np.asarray(inputs["w1"], np.float32) * n2[None, :]
    w3 = np.asarray(inputs["w3"], np.float32) * n2[None, :]
    w2 = np.asarray(inputs["w2"], np.float32)

    # qkv row permutation: for j<16 (q,k) parity split rows; v plain
    rows = np.zeros(3 * D, np.int64)
    for j in range(24):
        for col in range(P):
            if j < 16:
                h, r = divmod(col, 64)
                par, jp = divmod(r, 32)
                rows[j * P + col] = 128 * j + 64 * h + 2 * jp + par
            else:
                rows[j * P + col] = j * P + col
    wq_perm = w_qkv[rows, :]                   # (3D, D) row-permuted
    # wqkv8[p, c2, e, jf]  = wq_perm[jf, (2*c2+e)*128 + p] * SW
    wqkv8 = (wq_perm.T.reshape(4, 2, P, 3 * D).transpose(2, 0, 1, 3)
             * SW).astype(NP8)
    wqkv8 = np.ascontiguousarray(wqkv8)
    # wo8[p, c2, e, f] = w_out[f, (2*c2+e)*128 + p] * SW
    wo8 = (w_out.T.reshape(4, 2, P, D).transpose(2, 0, 1, 3) * SW).astype(NP8)
    wo8 = np.ascontiguousarray(wo8)
    # w13t[p, c, s, f]: s=0 -> w1[f, c*128+p], s=1 -> w3[f, c*128+p]*SH/2
    w1tt = w1.T.reshape(DC, P, DFF).transpose(1, 0, 2)
    w3tt = w3.T.reshape(DC, P, DFF).transpose(1, 0, 2) * (SH / 2)
    w13t = np.ascontiguousarray(
        np.stack([w1tt, w3tt], axis=2)).astype(np.float16)
    # w28[p, fp, e, f] = w2[f, (2*fp+e)*128 + p] * SW
    w28 = (w2.T.reshape(FC // 2, 2, P, D).transpose(2, 0, 1, 3)
           * SW).astype(NP8)
    w28 = np.ascontiguousarray(w28)
    return {"wqkv8": wqkv8, "wo8": wo8, "w13t": w13t, "w28": w28}


def build_in_maps(inputs):
    src = np.asarray(inputs["src"], dtype=np.float32)
    positions = np.asarray(inputs["positions"], dtype=np.float32)
    axis_scale = np.asarray(inputs["axis_scale"], np.float32)
    weights = _prep_weights(inputs)
    in_maps = []
    for c in range(N_CORES):
        b, h = c // 2, c % 2
        sp = src[b]
        pp = positions[b]
        if h == 1:  # own half first
            sp = np.concatenate([sp[TQ:], sp[:TQ]], axis=0)
            pp = np.concatenate([pp[TQ:], pp[:TQ]], axis=0)
        ct, st = _host_tables(pp, axis_scale)
        m = {"src": np.ascontiguousarray(sp), "cos_t": ct, "sin_t": st}
        m.update(weights)
        in_maps.append(m)
    return in_maps


def kernel(src, positions, w_qkv, w_out, norm1_w, norm2_w, w1, w2, w3,
           axis_scale):
    src = np.asarray(src, dtype=np.float32)
    B = src.shape[0]
    in_maps = build_in_maps(dict(
        src=src, positions=positions, w_qkv=w_qkv, w_out=w_out,
        norm1_w=norm1_w, norm2_w=norm2_w, w1=w1, w2=w2, w3=w3,
        axis_scale=axis_scale))
    nc = _get_nc()
    res = bass_utils.run_bass_kernel_spmd(nc, in_maps,
                                          core_ids=list(range(N_CORES)))
    outp = np.zeros((B, S, D), np.float32)
    for c in range(N_CORES):
        b, h = c // 2, c % 2
        outp[b, h * TQ:(h + 1) * TQ, :] = res.results[c]["out"]
    return outp


# revision 5
# speedup vs baseline: 1.0994x; 1.0083x over previous
"""Trainium2 Bass kernel: NeptuneTransformerEncoderLayer on 8 NeuronCores.

Sharding: batch(4) x seq-half(2) -> 8 cores, zero collectives.  Each core
computes K/V for its batch's full 2048 tokens and Q/attention/FFN for its
own 1024 tokens (host permutes src so own queries are rows [0:1024)).

v2: fp8e4 DoubleRow matmuls (0.5 cyc/row) for QKV, PV, out-proj and FFN
down-proj; f16 for scores and FFN up-proj (precision).  Weights are
host-side pre-transposed/pre-scaled/pre-quantized (no on-device weight
transposes).  The ACT-bound attention is split into two 512-query blocks
and the second block's attention is interleaved with the first block's
PE-bound FFN so both engines stay busy.  Scores PSUM tiles pair kc chunks
so exp keeps 1024-wide slices; PV emission is skewed one kc-pair behind
scores so the PE never waits on exp.
"""
import sys

for _p in ("/opt/trn_rl_repo", "/root/.axon_site/_ro/trn_rl_repo"):
    if _p not in sys.path:
        sys.path.insert(0, _p)

import numpy as np
import ml_dtypes

import concourse.bass as bass
import concourse.mybir as mybir
import concourse.tile as tile
from concourse import bacc
from concourse import bass_utils

F8 = mybir.dt.float8e4
F16 = mybir.dt.float16
F32 = mybir.dt.float32
AF = mybir.ActivationFunctionType
DR = mybir.MatmulPerfMode.DoubleRow
NP8 = ml_dtypes.float8_e4m3

P = 128
D = 1024            # d_model
DC = D // P         # 8 d-model chunks
NH = 16             # heads
HD = 64             # head dim
DFF = 4096
FC = DFF // P       # 32 ff chunks
S = 2048            # full sequence per batch
TQ = 1024           # query tokens per core
QB = 512            # query pipeline block
N_CORES = 8
EPS = 1e-5
BASE = 10000.0

SX = 8.0            # x_norm fp8 scale
SW = 64.0           # fp8 weight scale
SV = 8.0            # v fp8 scale
SH = 8.0            # hidden fp8 scale (via w3)
EXP_BIAS = 1.0      # exp(0.125*s - EXP_BIAS); score stats: max 4.36, rowmax>=1.2


def build_nc():
    nc = bacc.Bacc("TRN2", target_bir_lowering=False, debug=False,
                   num_devices=N_CORES)
    src = nc.dram_tensor("src", [S, D], F32, kind="ExternalInput")
    cos_t = nc.dram_tensor("cos_t", [P, S], F16, kind="ExternalInput")
    sin_t = nc.dram_tensor("sin_t", [P, S], F16, kind="ExternalInput")
    wqkv8 = nc.dram_tensor("wqkv8", [P, 4, 2, 3 * D], F8, kind="ExternalInput")
    wo8 = nc.dram_tensor("wo8", [P, 4, 2, D], F8, kind="ExternalInput")
    w13t = nc.dram_tensor("w13t", [P, DC, 2, DFF], F16, kind="ExternalInput")
    w28 = nc.dram_tensor("w28", [P, FC // 2, 2, D], F8, kind="ExternalInput")
    out = nc.dram_tensor("out", [TQ, D], F32, kind="ExternalOutput")

    with tile.TileContext(nc) as tc:
        emit(nc, tc, src, cos_t, sin_t, wqkv8, wo8, w13t, w28, out)
    nc.compile()
    return nc


def emit(nc, tc, src, cos_t, sin_t, wqkv8, wo8, w13t, w28, out):
    from contextlib import ExitStack

    ctx = ExitStack()
    with ctx:
        g_xnt = ctx.enter_context(ExitStack())   # XNT8/C2/S2/WQ8, freed post-B
        persist = ctx.enter_context(tc.tile_pool(name="persist", bufs=1))
        p_att = ctx.enter_context(tc.tile_pool(name="p_att", bufs=1))
        p_xnt = g_xnt.enter_context(
            tc.tile_pool(name="p_xnt", bufs=1, side="right"))
        p_qkv = ctx.enter_context(tc.tile_pool(name="p_qkv", bufs=1))

        XNT8 = p_xnt.tile([P, DC, S], F8)        # x_norm1.T fp8 (x8)
        WQ8 = p_xnt.tile([P, 4, 2, 3 * D], F8)   # qkv weights (24KB/p)
        C2 = p_xnt.tile([P, S], F16)
        S2 = p_xnt.tile([P, S], F16)
        QT = p_qkv.tile([P, DC, TQ], F16)        # roped q.T (true scale)
        KT = p_qkv.tile([P, DC, S], F16)         # roped k.T
        VA = p_qkv.tile([P, S // P, NH * 65], F8)   # v8 + ones col per head
        ATT = p_att.tile([P, DC, TQ], F8)        # attn out .T, fp8 (x8)
        WO8b = persist.tile([P, 4, 2, D], F8)
        eps_t = persist.tile([P, 1], F32)
        nc.vector.memset(eps_t[:], EPS)
        eps64_t = persist.tile([P, 1], F32)
        nc.vector.memset(eps64_t[:], EPS / (SX * SX))
        nbias_t = persist.tile([P, 1], F32)
        nc.vector.memset(nbias_t[:], -EXP_BIAS)

        nc.sync.dma_start(C2[:], cos_t[:])
        nc.sync.dma_start(S2[:], sin_t[:])
        nc.scalar.dma_start(WQ8[:], wqkv8[:])
        nc.scalar.dma_start(WO8b[:], wo8[:])

        va3 = VA.rearrange("p t (h c) -> p t h c", c=65)

        # long-lived working pools first (LIFO stack: closed last)
        phc = ctx.enter_context(tc.tile_pool(name="phc", bufs=2))
        phn = ctx.enter_context(tc.tile_pool(name="phn", bufs=1))
        pha_q = ctx.enter_context(tc.tile_pool(name="pha_q", bufs=1))
        phd = ctx.enter_context(tc.tile_pool(name="phd", bufs=2))
        phd_s = ctx.enter_context(tc.tile_pool(name="phd_s", bufs=4))
        phe_w = ctx.enter_context(tc.tile_pool(name="phe_w", bufs=3))
        phe = ctx.enter_context(tc.tile_pool(name="phe", bufs=2))
        phf = ctx.enter_context(tc.tile_pool(name="phf", bufs=2))
        phf_w = ctx.enter_context(tc.tile_pool(name="phf_w", bufs=2))
        # A/B working pools on top of the stack: freed before the overlap
        g_ab = ctx.enter_context(ExitStack())
        pha = g_ab.enter_context(tc.tile_pool(name="pha", bufs=2))
        pha_x = g_ab.enter_context(tc.tile_pool(name="pha_x", bufs=3))
        pha_t = g_ab.enter_context(tc.tile_pool(name="pha_t", bufs=2))
        pha_s = g_ab.enter_context(tc.tile_pool(name="pha_s", bufs=4))
        phb = g_ab.enter_context(tc.tile_pool(name="phb", bufs=2))

        st4 = [None]
        w13p = [None]

        def emit_a_ti(ti, _unused):
            if ti % 4 == 0:
                st4[0] = pha.tile([P, 4, D], F16, tag="src_in", name="st4")
                nc.gpsimd.dma_start(
                    st4[0][:], src.ap()[ti * P:(ti + 4) * P, :].rearrange(
                        "(g p) d -> p g d", g=4))
            st = st4[0][:, ti % 4, :]
            sqd = pha_q.tile([P, D], F16, tag="sqd")
            ssq = pha_s.tile([P, 1], F32, tag="ssq")
            nc.vector.scalar_tensor_tensor(
                sqd[:], st, 1.0, st, op0=mybir.AluOpType.mult,
                op1=mybir.AluOpType.mult, accum_out=ssq[:])
            # rms/SX = sqrt(ssq/(D*SX^2) + eps/SX^2) -> rinv = SX/rms
            rms = pha_s.tile([P, 1], F32, tag="rms")
            nc.scalar.activation(rms[:], ssq[:], AF.Sqrt,
                                 bias=eps64_t[:], scale=1.0 / (D * SX * SX))
            rinv = pha_s.tile([P, 1], F32, tag="rinv")
            nc.vector.reciprocal(rinv[:], rms[:])
            xn = pha_x.tile([P, D], F16, tag="xn")
            nc.vector.tensor_scalar_mul(xn[:], st, rinv[:])
            t16 = pha_t.tile([P, DC, P], F16, tag="t16")
            nc.sync.dma_start(t16[:], xn[:], transpose=True)
            nc.scalar.activation(XNT8[:, :, ti * P:(ti + 1) * P], t16[:],
                                 AF.Copy)

        def qk_slice(j, ts, ps_work):
            # DoubleRow qkv projection for 512 tokens of q (j<8) / k
            sl = slice(ts * 512, ts * 512 + 512)
            pk = ps_work.tile([P, 512], F32, tag="work")
            for c2 in range(4):
                nc.tensor.matmul(pk[:], WQ8[:, c2, :, j * P:(j + 1) * P],
                                 XNT8[:, 2 * c2:2 * c2 + 2, sl],
                                 start=(c2 == 0), stop=(c2 == 3),
                                 perf_mode=DR)
            # rope: evict to f16 (true scale), rotate pairs
            pk16 = phb.tile([P, 512], F16, tag="pk16")
            nc.vector.tensor_scalar_mul(pk16[:], pk[:], 1.0 / (SX * SW))
            aa = phb.tile([P, 512], F16, tag="aa")
            nc.vector.tensor_mul(aa[:], pk16[:], C2[:, sl])
            pp = phb.tile([P, 512], F16, tag="pp")
            nc.vector.tensor_mul(pp[:], pk16[:], S2[:, sl])
            bb = phb.tile([P, 512], F16, tag="bb")
            for h0 in (0, 64):
                nc.vector.tensor_copy(bb[h0:h0 + 32, :],
                                      pp[h0 + 32:h0 + 64, :])
                nc.vector.tensor_copy(bb[h0 + 32:h0 + 64, :],
                                      pp[h0:h0 + 32, :])
            dst = (QT[:, j, sl] if j < 8 else KT[:, j - 8, sl])
            nc.vector.tensor_add(dst, aa[:], bb[:])

        def emit_v_ti(ti, ps_work):
            for half in range(2):
                pvh = ps_work.tile([P, 512], F32, tag="work")
                for j4 in range(4):
                    j = 16 + half * 4 + j4
                    for c2 in range(4):
                        nc.tensor.matmul(
                            pvh[:, j4 * P:(j4 + 1) * P],
                            XNT8[:, 2 * c2:2 * c2 + 2, ti * P:(ti + 1) * P],
                            WQ8[:, c2, :, j * P:(j + 1) * P],
                            start=(c2 == 0), stop=(c2 == 3),
                            perf_mode=DR)
                hs = slice(half * 8, (half + 1) * 8)
                nc.vector.memset(va3[:, ti, hs, 64], 1.0)
                nc.vector.tensor_scalar_mul(
                    va3[:, ti, hs, 0:64],
                    pvh.rearrange("p (h c) -> p h c", c=64),
                    SV / (SX * SW))

        def attend(h, qb, ps_st, ps_pv, part=2):
            """Attention for head h, query block qb.  Yields after each
            emission chunk so the caller can interleave other engine work
            (part = number of chunks)."""
            j, hb = h // 2, 64 * (h % 2)
            qsl = slice(qb * QB, (qb + 1) * QB)
            ppv = ps_pv.tile([65, QB], F32, tag="ppv")
            pend = None   # skew: PV for pair kcp emitted after scores kcp+1
            for kcp in range(8):
                pst = ps_st.tile([P, 2, QB], F32, tag="pst")
                for e in range(2):
                    kc = 2 * kcp + e
                    nc.tensor.matmul(
                        pst[:, e, :],
                        KT[hb:hb + 64, j, kc * P:(kc + 1) * P],
                        QT[hb:hb + 64, j, qsl],
                        start=True, stop=True)
                pt8 = phc.tile([P, 2, QB], F8, tag="pt8")
                nc.scalar.activation(pt8[:], pst[:], AF.Exp,
                                     scale=0.125, bias=nbias_t[:])
                if pend is not None:
                    pk, pt = pend
                    nc.tensor.matmul(ppv[:],
                                     VA[:, 2 * pk:2 * pk + 2,
                                        65 * h:65 * h + 65],
                                     pt[:], start=(pk == 0), stop=False,
                                     perf_mode=DR)
                pend = (kcp, pt8)
                if part == 2 and kcp == 3:
                    yield
            pk, pt = pend
            nc.tensor.matmul(ppv[:],
                             VA[:, 2 * pk:2 * pk + 2, 65 * h:65 * h + 65],
                             pt[:], start=False, stop=True, perf_mode=DR)
            rec = phn.tile([1, QB], F32, tag="rec")
            nc.vector.reciprocal(rec[:], ppv[64:65, :])
            nrmS = phn.tile([64, QB], F32, tag="nrmS")
            nc.gpsimd.partition_broadcast(nrmS[:], rec[:])
            nc.vector.tensor_mul(ATT[hb:hb + 64, j, qsl], ppv[0:64, :],
                                 nrmS[:])
            yield

        def run_all(gen):
            for _ in gen:
                pass

        def emit_d_tb(tb, phd_ps, _unused):
            py = phd_ps.tile([P, D], F32, tag="py")
            for j in range(8):
                for c2 in range(4):
                    nc.tensor.matmul(py[:, j * P:(j + 1) * P],
                                     ATT[:, 2 * c2:2 * c2 + 2,
                                         tb * P:(tb + 1) * P],
                                     WO8b[:, c2, :, j * P:(j + 1) * P],
                                     start=(c2 == 0), stop=(c2 == 3),
                                     perf_mode=DR)
            srcq = phd.tile([P, D], F16, tag="srcq")
            nc.gpsimd.dma_start(srcq[:], src[tb * P:(tb + 1) * P, :])
            nc.vector.scalar_tensor_tensor(
                X2[:, tb, :], py[:], 1.0 / (SV * SW), srcq[:],
                op0=mybir.AluOpType.mult, op1=mybir.AluOpType.add)
            sqd2 = pha_q.tile([P, D], F16, tag="sqd")
            ssq = phd_s.tile([P, 1], F32, tag="ssq")
            nc.vector.scalar_tensor_tensor(
                sqd2[:], X2[:, tb, :], 1.0, X2[:, tb, :],
                op0=mybir.AluOpType.mult, op1=mybir.AluOpType.mult,
                accum_out=ssq[:])
            rms = phd_s.tile([P, 1], F32, tag="rms")
            nc.scalar.activation(rms[:], ssq[:], AF.Sqrt,
                                 bias=eps_t[:], scale=1.0 / D)
            rinv = phd_s.tile([P, 1], F32, tag="rinv")
            nc.vector.reciprocal(rinv[:], rms[:])
            xn2 = phd.tile([P, D], F16, tag="xn2")
            nc.vector.tensor_scalar_mul(xn2[:], X2[:, tb, :], rinv[:])
            nc.sync.dma_start(XN2T[:, :, tb * P:(tb + 1) * P], xn2[:],
                              transpose=True)

        def emit_e_fc(fc, qb, phe_ps, dge=None):
            qsl = slice(qb * QB, (qb + 1) * QB)
            w13c = phe_w.tile([P, DC, 2, P], F16, tag="w13c")
            (dge or nc.sync).dma_start(w13c[:],
                                       w13t[:, :, :, fc * P:(fc + 1) * P])
            pab = phe_ps.tile([P, 2, QB], F32, tag="pab")
            for s in range(2):
                for c in range(DC):
                    nc.tensor.matmul(pab[:, s, :], w13c[:, c, s, :],
                                     XN2T[:, c, qsl],
                                     start=(c == 0), stop=(c == DC - 1))
            # silu via tanh (same ACT table as Exp -> no table reloads):
            # 2*silu(x) = x*(1+tanh(x/2)); the 1/2 is folded into w3t.
            th = phe.tile([P, QB], F16, tag="th")
            nc.scalar.activation(th[:], pab[:, 0, :], AF.Tanh, scale=0.5)
            sa = phe.tile([P, QB], F16, tag="sa")
            nc.vector.scalar_tensor_tensor(
                sa[:], th[:], 1.0, pab[:, 0, :],
                op0=mybir.AluOpType.add, op1=mybir.AluOpType.mult)
            # HT8 holds one query block at a time (reused across qb passes)
            nc.vector.tensor_mul(HT8[:, fc, :], sa[:], pab[:, 1, :])

        def emit_e_fc_h(fc, phe_ps):
            # E for query block 0 in two 256-wide passes (1-bank PSUM tiles)
            for half in range(2):
                qsl = slice(half * 256, half * 256 + 256)
                w13c = phe_w.tile([P, DC, 2, P], F16, tag="w13c")
                if half == 0:
                    nc.sync.dma_start(w13c[:],
                                      w13t[:, :, :, fc * P:(fc + 1) * P])
                    w13k = w13c
                else:
                    w13k = w13p[0]
                w13p[0] = w13k
                pab = phe_ps.tile([P, 2, 256], F32, tag="pab")
                for s in range(2):
                    for c in range(DC):
                        nc.tensor.matmul(pab[:, s, :], w13k[:, c, s, :],
                                         XN2T[:, c, qsl],
                                         start=(c == 0), stop=(c == DC - 1))
                th = phe.tile([P, 256], F16, tag="th")
                nc.scalar.activation(th[:], pab[:, 0, :], AF.Tanh, scale=0.5)
                sa = phe.tile([P, 256], F16, tag="sa")
                nc.vector.scalar_tensor_tensor(
                    sa[:], th[:], 1.0, pab[:, 0, :],
                    op0=mybir.AluOpType.add, op1=mybir.AluOpType.mult)
                nc.vector.tensor_mul(HT8[:, fc, qsl], sa[:], pab[:, 1, :])

        def emit_f_qb(qb, phf_ps):
            # token-major w2 matmul: lhsT = hidden (stationary), rhs = w2
            # (moving) -> psum [tokens, features]; no transpose fold needed.
            pzts = [phf_ps.tile([P, D], F32, tag=f"pzt{tq}", name=f"pzt{tq}")
                    for tq in range(QB // P)]
            for j in range(8):
                w2j = phf_w.tile([P, FC // 2, 2, P], F8, tag="w2j")
                nc.sync.dma_start(w2j[:],
                                  w28[:, :, :, j * P:(j + 1) * P])
                for tq in range(QB // P):
                    for fp in range(FC // 2):
                        nc.tensor.matmul(
                            pzts[tq][:, j * P:(j + 1) * P],
                            HT8[:, 2 * fp:2 * fp + 2, tq * P:(tq + 1) * P],
                            w2j[:, fp, :, :],
                            start=(fp == 0), stop=(fp == FC // 2 - 1),
                            perf_mode=DR)
            for tq in range(QB // P):
                tb = qb * (QB // P) + tq
                nc.vector.scalar_tensor_tensor(
                    X2[:, tb, :], pzts[tq][:], 1.0 / (SH * SW), X2[:, tb, :],
                    op0=mybir.AluOpType.mult, op1=mybir.AluOpType.add)
                nc.sync.dma_start(out[tb * P:(tb + 1) * P, :], X2[:, tb, :])

        # ---------------- A + B + C(qb0), woven ----------------
        with tc.tile_pool(name="ps_work", bufs=2, space="PSUM") as ps_work, \
             tc.tile_pool(name="ps_st", bufs=2, space="PSUM") as ps_st, \
             tc.tile_pool(name="ps_pv", bufs=2, space="PSUM") as ps_pv:
            for ti in range(4):
                emit_a_ti(ti, None)
            emit_v_ti(0, ps_work)
            qk_slice(0, 0, ps_work)
            for ti in range(4, 8):
                emit_a_ti(ti, None)
                emit_v_ti(ti - 3, ps_work)
            qk_slice(0, 1, ps_work)
            qk_slice(8, 0, ps_work)
            for ti in range(8, 12):
                emit_a_ti(ti, None)
                emit_v_ti(ti - 3, ps_work)
            qk_slice(8, 1, ps_work)
            qk_slice(8, 2, ps_work)
            for ti in range(12, 16):
                emit_a_ti(ti, None)
                emit_v_ti(ti - 3, ps_work)
            qk_slice(8, 3, ps_work)
            for ti in range(13, S // P):
                emit_v_ti(ti, ps_work)
            run_all(attend(0, 0, ps_st, ps_pv))
            run_all(attend(1, 0, ps_st, ps_pv))
            for hp in range(1, 8):
                for jj in (hp, 8 + hp):
                    for ts in range(2 if jj < 8 else 4):
                        qk_slice(jj, ts, ps_work)
                run_all(attend(2 * hp, 0, ps_st, ps_pv))
                run_all(attend(2 * hp + 1, 0, ps_st, ps_pv))
        g_xnt.close()   # free XNT8 / WQ8 / C2 / S2
        g_ab.close()    # free A/B working pools

        # D/E/F big tiles go where XNT8/WQ8 were
        p_mid = ctx.enter_context(tc.tile_pool(name="p_mid", bufs=1,
                                               side="right"))
        X2 = p_mid.tile([P, TQ // P, D], F32)    # residual+output (t-major)
        XN2T = p_mid.tile([P, DC, TQ], F16)      # x_norm2.T
        HT8 = p_mid.tile([P, FC, QB], F8)        # swiglu hidden .T (x8)

        # ---------------- D(qb0) ----------------
        with tc.tile_pool(name="phd_ps", bufs=4, space="PSUM") as phd_ps:
            for tb in range(4):
                emit_d_tb(tb, phd_ps, None)

        # ---------- overlap: C(qb1) interleaved with E(qb0) ----------
        with tc.tile_pool(name="ps_st2", bufs=2, space="PSUM") as ps_st2, \
             tc.tile_pool(name="ps_pv2", bufs=2, space="PSUM") as ps_pv2, \
             tc.tile_pool(name="phe_ps", bufs=1, space="PSUM") as phe_ps:
            for h in range(NH):
                gen = attend(h, 1, ps_st2, ps_pv2)
                next(gen)
                emit_e_fc(2 * h, 0, phe_ps)
                run_all(gen)
                emit_e_fc(2 * h + 1, 0, phe_ps)
        # p_qkv (QT/KT/VA) stays allocated; fits alongside p_mid

        # ---------------- D(qb1), then F(qb0) hiding D1's latency ------
        with tc.tile_pool(name="phd_ps", bufs=4, space="PSUM") as phd_ps:
            for tb in range(4, 8):
                emit_d_tb(tb, phd_ps, None)
        with tc.tile_pool(name="phf_ps", bufs=1, space="PSUM") as phf_ps:
            emit_f_qb(0, phf_ps)

        # ---------------- E(qb1) + F(qb1) ----------------
        with tc.tile_pool(name="phe_ps2", bufs=3, space="PSUM") as phe_ps2:
            for fc in range(FC):
                emit_e_fc(fc, 1, phe_ps2)
        with tc.tile_pool(name="phf_ps", bufs=1, space="PSUM") as phf_ps:
            emit_f_qb(1, phf_ps)


_NC_CACHE = None


def _get_nc():
    global _NC_CACHE
    if _NC_CACHE is None:
        _NC_CACHE = build_nc()
    return _NC_CACHE


def _host_tables(positions_b, axis_scale):
    """Build parity-split fp16 cos/sin tables (128, S) for one batch."""
    coord = positions_b * axis_scale[None, :]              # (S, 4)
    invf = BASE ** (-(np.arange(0, 16, 2, dtype=np.float32) / 16.0))  # (8,)
    ang = coord[:, :, None] * invf[None, None, :]          # (S, 4, 8)
    ang = ang.reshape(S, 32).T                             # (32, S): r = 8a + j
    cos64 = np.concatenate([np.cos(ang), np.cos(ang)], axis=0)   # (64, S)
    sin64 = np.concatenate([np.sin(ang), -np.sin(ang)], axis=0)  # (64, S)
    c128 = np.concatenate([cos64, cos64], axis=0).astype(np.float16)
    s128 = np.concatenate([sin64, sin64], axis=0).astype(np.float16)
    return c128, s128


def _prep_weights(inputs):
    """Pre-transpose / permute / scale / quantize all weights on the host."""
    n1 = np.asarray(inputs["norm1_w"], np.float32)
    n2 = np.asarray(inputs["norm2_w"], np.float32)
    w_qkv = np.asarray(inputs["w_qkv"], np.float32) * n1[None, :]
    w_out = np.asarray(inputs["w_out"], np.float32)
    w1 = np.asarray(inputs["w1"], np.float32) * n2[None, :]
    w3 = np.asarray(inputs["w3"], np.float32) * n2[None, :]
    w2 = np.asarray(inputs["w2"], np.float32)

    # qkv row permutation: for j<16 (q,k) parity split rows; v plain
    rows = np.zeros(3 * D, np.int64)
    for j in range(24):
        for col in range(P):
            if j < 16:
                h, r = divmod(col, 64)
                par, jp = divmod(r, 32)
                rows[j * P + col] = 128 * j + 64 * h + 2 * jp + par
            else:
                rows[j * P + col] = j * P + col
    wq_perm = w_qkv[rows, :]                   # (3D, D) row-permuted
    # wqkv8[p, c2, e, jf]  = wq_perm[jf, (2*c2+e)*128 + p] * SW
    wqkv8 = (wq_perm.T.reshape(4, 2, P, 3 * D).transpose(2, 0, 1, 3)
             * SW).astype(NP8)
    wqkv8 = np.ascontiguousarray(wqkv8)
    # wo8[p, c2, e, f] = w_out[f, (2*c2+e)*128 + p] * SW
    wo8 = (w_out.T.reshape(4, 2, P, D).transpose(2, 0, 1, 3) * SW).astype(NP8)
    wo8 = np.ascontiguousarray(wo8)
    # w13t[p, c, s, f]: s=0 -> w1[f, c*128+p], s=1 -> w3[f, c*128+p]*SH/2
    w1tt = w1.T.reshape(DC, P, DFF).transpose(1, 0, 2)
    w3tt = w3.T.reshape(DC, P, DFF).transpose(1, 0, 2) * (SH / 2)
    w13t = np.ascontiguousarray(
        np.stack([w1tt, w3tt], axis=2)).astype(np.float16)
    # w28[p, fp, e, f] = w2[f, (2*fp+e)*128 + p] * SW
    w28 = (w2.T.reshape(FC // 2, 2, P, D).transpose(2, 0, 1, 3)
           * SW).astype(NP8)
    w28 = np.ascontiguousarray(w28)
    return {"wqkv8": wqkv8, "wo8": wo8, "w13t": w13t, "w28": w28}


def build_in_maps(inputs):
    src = np.asarray(inputs["src"], dtype=np.float32)
    positions = np.asarray(inputs["positions"], dtype=np.float32)
    axis_scale = np.asarray(inputs["axis_scale"], np.float32)
    weights = _prep_weights(inputs)
    in_maps = []
    for c in range(N_CORES):
        b, h = c // 2, c % 2
        sp = src[b]
        pp = positions[b]
        if h == 1:  # own half first
            sp = np.concatenate([sp[TQ:], sp[:TQ]], axis=0)
            pp = np.concatenate([pp[TQ:], pp[:TQ]], axis=0)
        ct, st = _host_tables(pp, axis_scale)
        m = {"src": np.ascontiguousarray(sp), "cos_t": ct, "sin_t": st}
        m.update(weights)
        in_maps.append(m)
    return in_maps


def kernel(src, positions, w_qkv, w_out, norm1_w, norm2_w, w1, w2, w3,
           axis_scale):
    src = np.asarray(src, dtype=np.float32)
    B = src.shape[0]
    in_maps = build_in_maps(dict(
        src=src, positions=positions, w_qkv=w_qkv, w_out=w_out,
        norm1_w=norm1_w, norm2_w=norm2_w, w1=w1, w2=w2, w3=w3,
        axis_scale=axis_scale))
    nc = _get_nc()
    res = bass_utils.run_bass_kernel_spmd(nc, in_maps,
                                          core_ids=list(range(N_CORES)))
    outp = np.zeros((B, S, D), np.float32)
    for c in range(N_CORES):
        b, h = c // 2, c % 2
        outp[b, h * TQ:(h + 1) * TQ, :] = res.results[c]["out"]
    return outp
